# revision 3
# baseline (speedup 1.0000x reference)
"""Trainium2 Bass kernel for GuidedImplicitPointSampler KNN (top-8 + occupancy mask).

Strategy:
  - Shard the N=32768 query points across 8 NeuronCores (4096 each); every core
    holds the full M=16384 target cloud.
  - Per core, compute s[n,m] = 2*q_n.k_m - |k_m|^2 on the PE as a K=4 matmul
    (rows: 2kx,2ky,2kz,-|k|^2 moving; qx,qy,qz,1 stationary).  Since
    d2[n,m] = |q_n|^2 - s[n,m] and |q_n|^2 is constant per row, the 8 nearest
    targets are exactly the 8 LARGEST s values -> hardware top-8 (`nc.vector.max`)
    straight out of PSUM, no full distance matrix ever materialized.
  - K=4 uses only 4 of 128 PE rows, so 4 independent matmuls are packed into
    row-groups 0/32/64/96 via tile_position and run concurrently.
  - Epilogue on the tiny [128, 8] results: d = sqrt(max(q2 - s, 0)), zero rows
    whose nearest distance <= 0.25.
"""

import numpy as np

N = 32768
M = 16384
KNN = 8
OCC_RADIUS = 0.25
N_CORES = 8
NPC = N // N_CORES        # 4096 queries per core
RT = NPC // 128           # 32 row-tiles of 128 queries
CHUNK = 512               # matmul moving free dim (one PSUM bank)
PACK = 4                  # concurrent matmuls in PE row-groups
GROUP = CHUNK * PACK      # 2048 targets per PSUM tile
NGROUP = M // GROUP       # 8 groups per row-tile

_CACHE = {}


def _build(reps=1, mmdt="f16x3"):
    key = ("nc", reps, mmdt)
    if key in _CACHE:
        return _CACHE[key]
    from concourse import bacc, tile, mybir

    dt = mybir.dt
    # mmdt selects the PE path for s = 2q.k - |k|^2 (all run 1 cycle/row
    # except plain f32 which is 4):
    #   "f16k11": fp16 hi/lo split folded into ONE K=11 matmul:
    #            lhsT col = [qh(3), ql(3), qh(3), 1, 1],
    #            rhs col = [kh(3), kh(3), kl(3), -k2h, -k2l]  (error ~2^-22,
    #            same PE rate as K=4 since K<=32 fits one row-group).
    #   "f16x3": same terms as 3 separate accumulating matmuls (slower PE).
    #   "f32r":  fp32 bits at full PE rate, reduced internal precision (~2^-18).
    #   "f32":   exact fp32, 4x slower PE.
    mdt = {"f32r": dt.float32r, "f32": dt.float32, "f16x3": dt.float16,
           "f16k11": dt.float16}[mmdt]
    KDIM = 11 if mmdt == "f16k11" else 4
    nc = bacc.Bacc("TRN2", target_bir_lowering=False, debug=False,
                   num_devices=N_CORES)

    if mmdt == "f16x3":
        lhsA_d = nc.dram_tensor("lhsA", [4, NPC], mdt, kind="ExternalInput")
        lhsB_d = nc.dram_tensor("lhsB", [4, NPC], mdt, kind="ExternalInput")
        rhsH_d = nc.dram_tensor("rhsH", [4, M], mdt, kind="ExternalInput")
        rhsL_d = nc.dram_tensor("rhsL", [4, M], mdt, kind="ExternalInput")
    else:
        lhsT_d = nc.dram_tensor("lhsT", [KDIM, NPC], mdt, kind="ExternalInput")
        rhs_d = nc.dram_tensor("rhs", [KDIM, M], mdt, kind="ExternalInput")
    q2_d = nc.dram_tensor("q2rep", [128, RT * KNN], dt.float32,
                          kind="ExternalInput")
    out_d = nc.dram_tensor("out", [NPC, KNN], dt.float32, kind="ExternalOutput")

    with tile.TileContext(nc) as tc:
        with (
            tc.tile_pool(name="const", bufs=1) as constp,
            tc.tile_pool(name="psum", bufs=2, space="PSUM") as psump,
            tc.tile_pool(name="cand", bufs=3) as candp,
            tc.tile_pool(name="fin", bufs=1) as finp,
        ):
            q2_sb = constp.tile([128, RT * KNN], dt.float32)
            nc.sync.dma_start(out=q2_sb[:, :], in_=q2_d[:, :])
            # Replicate operands into each PE row-group's partition range.
            if mmdt == "f16x3":
                lhsA_sb = constp.tile([128, NPC], mdt)
                lhsB_sb = constp.tile([128, NPC], mdt)
                rhsH_sb = constp.tile([128, M], mdt)
                rhsL_sb = constp.tile([128, M], mdt)
                for i in range(PACK):
                    p = 32 * i
                    nc.sync.dma_start(out=lhsA_sb[p:p + 4, :], in_=lhsA_d[:, :])
                    nc.sync.dma_start(out=lhsB_sb[p:p + 4, :], in_=lhsB_d[:, :])
                    nc.sync.dma_start(out=rhsH_sb[p:p + 4, :], in_=rhsH_d[:, :])
                    nc.sync.dma_start(out=rhsL_sb[p:p + 4, :], in_=rhsL_d[:, :])
            else:
                rhs_sb = constp.tile([128, M], mdt)
                lhs_sb = constp.tile([128, NPC], mdt)
                for i in range(PACK):
                    nc.sync.dma_start(out=rhs_sb[32 * i:32 * i + KDIM, :], in_=rhs_d[:, :])
                    nc.sync.dma_start(out=lhs_sb[32 * i:32 * i + KDIM, :], in_=lhsT_d[:, :])

            s8_all = finp.tile([128, RT * KNN], dt.float32)
            for _rep in range(reps):
                for t in range(RT):
                    cands = candp.tile([128, NGROUP * KNN], dt.float32, tag="cands")
                    for g in range(NGROUP):
                        ps = psump.tile([128, GROUP], dt.float32, tag="ps")
                        for i in range(PACK):
                            c = g * PACK + i
                            p = 32 * i
                            tcol = slice(t * 128, (t + 1) * 128)
                            ccol = slice(c * CHUNK, (c + 1) * CHUNK)
                            pslice = ps[:, i * CHUNK:(i + 1) * CHUNK]
                            if mmdt == "f16x3":
                                for j, (lsb, rsb) in enumerate(
                                        [(lhsA_sb, rhsH_sb), (lhsB_sb, rhsH_sb),
                                         (lhsA_sb, rhsL_sb)]):
                                    nc.tensor.matmul(
                                        out=pslice,
                                        lhsT=lsb[p:p + 4, tcol],
                                        rhs=rsb[p:p + 4, ccol],
                                        start=(j == 0), stop=(j == 2),
                                        tile_position=(p, 0),
                                    )
                            else:
                                nc.tensor.matmul(
                                    out=pslice,
                                    lhsT=lhs_sb[p:p + KDIM, tcol],
                                    rhs=rhs_sb[p:p + KDIM, ccol],
                                    start=True, stop=True,
                                    tile_position=(p, 0),
                                )
                        nc.vector.max(out=cands[:, g * KNN:(g + 1) * KNN], in_=ps[:, :])
                    nc.vector.max(out=s8_all[:, t * KNN:(t + 1) * KNN], in_=cands[:, :])

            # Epilogue: d = sqrt(max(q2 - s, 0)); zero rows with min dist <= 0.25
            d2 = finp.tile([128, RT * KNN], dt.float32)
            nc.vector.tensor_sub(d2[:, :], q2_sb[:, :], s8_all[:, :])
            nc.vector.tensor_scalar_max(d2[:, :], d2[:, :], 0.0)
            dst = finp.tile([128, RT * KNN], dt.float32)
            nc.scalar.activation(dst[:, :], d2[:, :],
                                 mybir.ActivationFunctionType.Sqrt)
            good = finp.tile([128, RT], dt.float32)
            nc.vector.tensor_scalar(good[:, :], dst[:, 0:RT * KNN:KNN],
                                    OCC_RADIUS, None, mybir.AluOpType.is_gt)
            res = finp.tile([128, RT * KNN], dt.float32)
            nc.vector.tensor_tensor(
                res[:, :].rearrange("p (t j) -> p t j", j=KNN),
                dst[:, :].rearrange("p (t j) -> p t j", j=KNN),
                good[:, :, None].broadcast_to([128, RT, KNN]),
                mybir.AluOpType.mult,
            )
            nc.sync.dma_start(
                out=out_d.ap().rearrange("(t p) j -> p t j", p=128),
                in_=res[:, :].rearrange("p (t j) -> p t j", j=KNN),
            )

    nc.compile()
    _CACHE[key] = nc
    return nc


def _prep_in_maps(to_filter, target_coords, mmdt="f16x3"):
    q = np.ascontiguousarray(np.asarray(to_filter, dtype=np.float32)[:, :3])
    k = np.ascontiguousarray(np.asarray(target_coords, dtype=np.float32)[:, :3])
    q2 = np.sum(q * q, axis=1, dtype=np.float32)
    k2 = np.sum(k * k, axis=1, dtype=np.float32)
    in_maps = []
    if mmdt == "f16x3":
        qh = q.astype(np.float16)
        ql = (q - qh.astype(np.float32)).astype(np.float16)
        k2x = 2.0 * k.T
        kh = k2x.astype(np.float16)
        kl = (k2x - kh.astype(np.float32)).astype(np.float16)
        k2h = k2.astype(np.float16)
        k2l = (k2 - k2h.astype(np.float32)).astype(np.float16)
        lhsA = np.empty((4, N), np.float16)
        lhsA[0:3] = qh.T
        lhsA[3] = 1.0
        lhsB = np.empty((4, N), np.float16)
        lhsB[0:3] = ql.T
        lhsB[3] = 0.0
        rhsH = np.empty((4, M), np.float16)
        rhsH[0:3] = kh
        rhsH[3] = -k2h
        rhsL = np.empty((4, M), np.float16)
        rhsL[0:3] = kl
        rhsL[3] = -k2l
        for c in range(N_CORES):
            q2c = q2[c * NPC:(c + 1) * NPC].reshape(RT, 128).T
            in_maps.append({
                "lhsA": np.ascontiguousarray(lhsA[:, c * NPC:(c + 1) * NPC]),
                "lhsB": np.ascontiguousarray(lhsB[:, c * NPC:(c + 1) * NPC]),
                "rhsH": rhsH, "rhsL": rhsL,
                "q2rep": np.ascontiguousarray(np.repeat(q2c, KNN, axis=1)),
            })
        return in_maps
    if mmdt == "f16k11":
        # s = (qh+ql)·(2k)h + qh·(2k)l - k2h - k2l  (missing ql·(2k)l ~ 2^-22)
        qh = q.astype(np.float16)
        ql = (q - qh.astype(np.float32)).astype(np.float16)
        k2x = 2.0 * k.T
        kh = k2x.astype(np.float16)
        kl = (k2x - kh.astype(np.float32)).astype(np.float16)
        k2h = k2.astype(np.float16)
        k2l = (k2 - k2h.astype(np.float32)).astype(np.float16)
        lhsT = np.empty((11, N), np.float16)
        lhsT[0:3] = qh.T
        lhsT[3:6] = ql.T
        lhsT[6:9] = qh.T
        lhsT[9] = 1.0
        lhsT[10] = 1.0
        rhs = np.empty((11, M), np.float16)
        rhs[0:3] = kh
        rhs[3:6] = kh
        rhs[6:9] = kl
        rhs[9] = -k2h
        rhs[10] = -k2l
        for c in range(N_CORES):
            q2c = q2[c * NPC:(c + 1) * NPC].reshape(RT, 128).T
            in_maps.append({
                "lhsT": np.ascontiguousarray(lhsT[:, c * NPC:(c + 1) * NPC]),
                "rhs": rhs,
                "q2rep": np.ascontiguousarray(np.repeat(q2c, KNN, axis=1)),
            })
        return in_maps
    lhsT_full = np.empty((4, N), np.float32)
    lhsT_full[0:3] = q.T
    lhsT_full[3] = 1.0
    rhs = np.empty((4, M), np.float32)
    rhs[0:3] = 2.0 * k.T
    rhs[3] = -k2
    for c in range(N_CORES):
        q2c = q2[c * NPC:(c + 1) * NPC].reshape(RT, 128).T  # [128, RT]
        q2rep = np.repeat(q2c, KNN, axis=1)                 # [128, RT*KNN]
        in_maps.append({
            "lhsT": np.ascontiguousarray(lhsT_full[:, c * NPC:(c + 1) * NPC]),
            "rhs": rhs,
            "q2rep": np.ascontiguousarray(q2rep),
        })
    return in_maps


def _run(to_filter, target_coords, trace=False, mmdt="f16x3"):
    from concourse import bass_utils

    nc = _build(mmdt=mmdt)
    in_maps = _prep_in_maps(to_filter, target_coords, mmdt=mmdt)
    res = bass_utils.run_bass_kernel_spmd(
        nc, in_maps, core_ids=list(range(N_CORES)), trace=trace,
    )
    out = np.concatenate([r["out"] for r in res.results], axis=0)
    return out, res


def kernel(to_filter, target_coords):
    out, _ = _run(to_filter, target_coords)
    return out



# revision 4
# speedup vs baseline: 6.5214x; 6.5214x over previous
"""Trainium2 Bass kernel for GuidedImplicitPointSampler KNN (top-8 + occupancy mask).

Strategy (pruned, exact):
  - Host groups the N=32768 queries into 256 spatial tiles of 128 (k-d median
    splits) and, per tile, builds a provably sufficient candidate subset of the
    M=16384 targets from grid cell COUNTS only (no host distance math):
      * ub8(q): walk cell offsets sorted by worst-case point-to-point distance
        until >= 9 targets are guaranteed; d8(q) <= ub8(q).  Two-level grid
        (coarse 0.30 everywhere, fine 0.06 refine in dense regions).
      * tile candidates: every target within R_t = max_q ub8(q) of the tile's
        bbox (cylinder-trimmed cell ranges; superset by construction).
    The device then computes exact distances + top-8 over the candidates, so
    the result equals brute force (candidates contain each query's true 8-NN
    and its nearest target, which also decides the 0.25 occupancy mask).
  - Tiles are dealt to 8 cores x 32 slots (sorted by size, groups of 8) so the
    SPMD program sees identical slot capacities; blocks are sentinel-padded.
  - Per slot: s[n,m] = 2q.k - |k|^2 on the PE as one K=11 fp16 hi/lo matmul
    (error ~2^-22), chunks of <=512 columns packed 4-wide into PE row groups
    0/32/64/96; top-8 via hardware MAX8 straight out of PSUM.
  - Epilogue: d = sqrt(max(q2 - s, 0)), zero rows whose nearest dist <= 0.25;
    host scatters rows back to the original query order.
"""

import numpy as np

N = 32768
M = 16384
KNN = 8
OCC_RADIUS = 0.25
N_CORES = 8
TILE = 128
NTILES = N // TILE            # 256
RT = NTILES // N_CORES        # 32 slots per core
CHUNK = 512                   # matmul moving free dim (one PSUM bank)
PACK = 4                      # concurrent matmuls in PE row-groups
GROUP = CHUNK * PACK          # 2048 target cols per PSUM tile
KDIM = 11
KSAFE = 9
SENTINEL = 60.0

_CACHE = {}


# ---------------------------------------------------------------------------
# Host-side pruning plan (grid counting only, no host distance computations)
# ---------------------------------------------------------------------------

def _cell_counts(pts, lo, h, n):
    ci = np.clip(((pts - lo) / h).astype(np.int64), 0, n - 1)
    cnt = np.zeros((n, n, n), np.int32)
    np.add.at(cnt, (ci[:, 0], ci[:, 1], ci[:, 2]), 1)
    return ci, cnt


def _sorted_offsets(max_cells):
    r = np.arange(-max_cells, max_cells + 1)
    X, Y, Z = np.meshgrid(r, r, r, indexing="ij")
    off = np.stack([X.ravel(), Y.ravel(), Z.ravel()], 1)
    wd = np.sqrt(((np.abs(off) + 1) ** 2).sum(1).astype(np.float64))
    o = np.argsort(wd, kind="stable")
    return off[o], wd[o]


def _walk_ub(cells, cnt, n, offsets, wdist, h, ksafe, chunk=512):
    """Per cell row: smallest wdist*h whose offset-prefix covers >= ksafe targets."""
    U = len(cells)
    ub = np.full(U, np.inf)
    acc = np.zeros(U, np.int64)
    alive = np.arange(U)
    for s in range(0, len(offsets), chunk):
        if len(alive) == 0:
            break
        offs = offsets[s:s + chunk]
        cc = cells[alive][:, None, :] + offs[None, :, :]
        ok = ((cc >= 0) & (cc < n)).all(2)
        cc = np.clip(cc, 0, n - 1)
        counts = cnt[cc[..., 0], cc[..., 1], cc[..., 2]] * ok
        ccum = counts.cumsum(1) + acc[alive][:, None]
        crossed = ccum >= ksafe
        hit = crossed.any(1)
        first = np.argmax(crossed, 1)
        hit_rows = alive[hit]
        ub[hit_rows] = wdist[s + first[hit]] * h
        acc[alive] = ccum[:, -1]
        alive = alive[~hit]
    return ub


def _kd_tiles(q, leaf=TILE):
    out = []

    def rec(ids):
        if len(ids) <= leaf:
            out.append(ids)
            return
        pts = q[ids]
        d = np.argmax(pts.max(0) - pts.min(0))
        half = ((len(ids) // 2) // leaf) * leaf
        o = np.argsort(pts[:, d], kind="stable")
        rec(ids[o[:half]])
        rec(ids[o[half:]])

    rec(np.arange(len(q)))
    return np.concatenate(out)


def _build_plan(q, k, hc=0.30, hf=0.06, hg=0.15, refine_thr=0.9, safety=1.02):
    lo = float(min(q.min(), k.min())) - 1e-4
    hi = float(max(q.max(), k.max())) + 1e-4

    # per-query upper bound on the 8-NN distance (coarse, then fine refine)
    nc_ = int(np.ceil((hi - lo) / hc))
    qic = np.clip(((q - lo) / hc).astype(np.int64), 0, nc_ - 1)
    _, cntc = _cell_counts(k, lo, hc, nc_)
    cells_u, inv = np.unique(qic, axis=0, return_inverse=True)
    offc, wdc = _sorted_offsets(nc_)
    ub = _walk_ub(cells_u, cntc, nc_, offc, wdc, hc, KSAFE)[inv]
    assert np.isfinite(ub).all()

    nf = int(np.ceil((hi - lo) / hf))
    qif = np.clip(((q - lo) / hf).astype(np.int64), 0, nf - 1)
    _, cntf = _cell_counts(k, lo, hf, nf)
    ref = ub <= refine_thr
    if ref.any():
        cells_f, invf = np.unique(qif[ref], axis=0, return_inverse=True)
        offf, wdf = _sorted_offsets(int(np.ceil(refine_thr / hf)) + 1)
        ubf = _walk_ub(cells_f, cntf, nf, offf, wdf, hf, KSAFE)[invf]
        idx = np.nonzero(ref)[0]
        better = ubf < ub[ref]
        ub[idx[better]] = ubf[better]
    ub *= safety

    perm = _kd_tiles(q)

    # gather CSR over the gather grid
    ng = int(np.ceil((hi - lo) / hg))
    kig = np.clip(((k - lo) / hg).astype(np.int64), 0, ng - 1)
    kcell = (kig[:, 0] * ng + kig[:, 1]) * ng + kig[:, 2]
    korder = np.argsort(kcell, kind="stable")
    kcs = kcell[korder]
    starts = np.searchsorted(kcs, np.arange(ng * ng * ng))
    ends = np.searchsorted(kcs, np.arange(ng * ng * ng), side="right")

    cand_lists = []
    for t in range(NTILES):
        qs = perm[t * TILE:(t + 1) * TILE]
        R = float(ub[qs].max())
        R2 = R * R
        blo, bhi = q[qs].min(0), q[qs].max(0)
        a = np.maximum(((blo - R - lo) / hg).astype(np.int64), 0)
        b = np.minimum(((bhi + R - lo) / hg).astype(np.int64), ng - 1)
        parts = []
        for ix in range(a[0], b[0] + 1):
            cx0, cx1 = lo + ix * hg, lo + (ix + 1) * hg
            dx = max(blo[0] - cx1, cx0 - bhi[0], 0.0)
            if dx * dx > R2:
                continue
            for iy in range(a[1], b[1] + 1):
                cy0, cy1 = lo + iy * hg, lo + (iy + 1) * hg
                dy = max(blo[1] - cy1, cy0 - bhi[1], 0.0)
                dxy2 = dx * dx + dy * dy
                if dxy2 > R2:
                    continue
                zh = float(np.sqrt(R2 - dxy2))
                z0 = max(int((blo[2] - zh - lo) / hg), 0)
                z1 = min(int((bhi[2] + zh - lo) / hg), ng - 1)
                base = (ix * ng + iy) * ng
                s, e = starts[base + z0], ends[base + z1]
                if e > s:
                    parts.append(korder[s:e])
        cand_lists.append(np.concatenate(parts) if parts
                          else np.empty(0, np.int64))

    # deal tiles to cores/slots: sort by size desc, slot i <- tiles [8i, 8i+8)
    sizes = np.array([len(c) for c in cand_lists])
    order = np.argsort(-sizes, kind="stable")
    tile_of = order.reshape(RT, N_CORES)            # [slot, core]
    caps = np.empty(RT, np.int64)
    for i in range(RT):
        caps[i] = max(int(np.ceil(sizes[tile_of[i]].max() / 128.0)) * 128, 128)
    return perm, cand_lists, tile_of, caps


def _f16_split(x):
    h = x.astype(np.float16)
    l = (x - h.astype(np.float32)).astype(np.float16)
    return h, l


def _rhs_block(kpts):
    """[11, C] fp16 block: rows = [(2k)h x3, (2k)h x3, (2k)l x3, -|k|2h, -|k|2l]."""
    k2 = (kpts * kpts).sum(1, dtype=np.float32)
    kh, kl = _f16_split(2.0 * kpts.T)
    k2h, k2l = _f16_split(k2)
    blk = np.empty((KDIM, len(kpts)), np.float16)
    blk[0:3] = kh
    blk[3:6] = kh
    blk[6:9] = kl
    blk[9] = -k2h
    blk[10] = -k2l
    return blk


def _prep_pruned(to_filter, target_coords):
    q = np.ascontiguousarray(np.asarray(to_filter, np.float32)[:, :3])
    k = np.ascontiguousarray(np.asarray(target_coords, np.float32)[:, :3])
    perm, cand_lists, tile_of, caps = _build_plan(q, k)
    capsum = int(caps.sum())
    offs = np.concatenate([[0], np.cumsum(caps)]).astype(np.int64)

    sent = np.full(3, SENTINEL, np.float32)
    in_maps = []
    row_query = np.empty((N_CORES, RT * TILE), np.int64)
    for c in range(N_CORES):
        qsel = np.empty(RT * TILE, np.int64)
        rhs_all = np.empty((KDIM, capsum), np.float16)
        for i in range(RT):
            t = tile_of[i, c]
            qsel[i * TILE:(i + 1) * TILE] = perm[t * TILE:(t + 1) * TILE]
            cand = cand_lists[t]
            cap = int(caps[i])
            kp = np.empty((cap, 3), np.float32)
            kp[:len(cand)] = k[cand]
            kp[len(cand):] = sent
            rhs_all[:, offs[i]:offs[i + 1]] = _rhs_block(kp)
        row_query[c] = qsel
        qc = q[qsel]
        q2 = (qc * qc).sum(1, dtype=np.float32)
        qh, ql = _f16_split(qc.T)
        lhsT = np.empty((KDIM, RT * TILE), np.float16)
        lhsT[0:3] = qh
        lhsT[3:6] = ql
        lhsT[6:9] = qh
        lhsT[9] = 1.0
        lhsT[10] = 1.0
        q2c = q2.reshape(RT, TILE).T                       # [128, RT]
        in_maps.append({
            "lhsT": np.ascontiguousarray(lhsT),
            "rhs_all": np.ascontiguousarray(rhs_all),
            "q2rep": np.ascontiguousarray(np.repeat(q2c, KNN, axis=1)),
        })
    return in_maps, row_query, tuple(int(x) for x in caps)


def _build_pruned(caps):
    key = ("pruned", caps)
    if key in _CACHE:
        return _CACHE[key]
    from concourse import bacc, tile, mybir

    dt = mybir.dt
    capsum = sum(caps)
    npc = RT * TILE
    nc = bacc.Bacc("TRN2", target_bir_lowering=False, debug=False,
                   num_devices=N_CORES)

    lhsT_d = nc.dram_tensor("lhsT", [KDIM, npc], dt.float16, kind="ExternalInput")
    rhs_d = nc.dram_tensor("rhs_all", [KDIM, capsum], dt.float16,
                           kind="ExternalInput")
    q2_d = nc.dram_tensor("q2rep", [128, RT * KNN], dt.float32,
                          kind="ExternalInput")
    out_d = nc.dram_tensor("out", [npc, KNN], dt.float32, kind="ExternalOutput")

    maxcap = max(caps)
    with tile.TileContext(nc) as tc:
        with (
            tc.tile_pool(name="const", bufs=1) as constp,
            tc.tile_pool(name="rhs", bufs=3) as rhsp,
            tc.tile_pool(name="psum", bufs=2, space="PSUM") as psump,
            tc.tile_pool(name="cand", bufs=2) as candp,
            tc.tile_pool(name="fin", bufs=1) as finp,
        ):
            q2_sb = constp.tile([128, RT * KNN], dt.float32)
            nc.sync.dma_start(out=q2_sb[:, :], in_=q2_d[:, :])
            lhs_sb = constp.tile([128, npc], dt.float16)
            for j in range(PACK):
                nc.sync.dma_start(out=lhs_sb[32 * j:32 * j + KDIM, :],
                                  in_=lhsT_d[:, :])

            s8_all = finp.tile([128, RT * KNN], dt.float32)
            off = 0
            for i in range(RT):
                cap = caps[i]
                ngroups = (cap + GROUP - 1) // GROUP
                rhs_sb = rhsp.tile([128, maxcap], dt.float16, tag="rhs")
                # chunk j of group g lives in PE row-group j (rows 32j..32j+10)
                for g in range(ngroups):
                    g0 = g * GROUP
                    gw = min(GROUP, cap - g0)
                    for j in range((gw + CHUNK - 1) // CHUNK):
                        c0 = g0 + j * CHUNK
                        w = min(CHUNK, cap - c0)
                        nc.sync.dma_start(
                            out=rhs_sb[32 * j:32 * j + KDIM, c0:c0 + w],
                            in_=rhs_d[:, off + c0:off + c0 + w])
                cands = None
                if ngroups > 1:
                    cands = candp.tile([128, ngroups * KNN], dt.float32,
                                       tag="cands")
                tcol = slice(i * TILE, (i + 1) * TILE)
                for g in range(ngroups):
                    g0 = g * GROUP
                    gw = min(GROUP, cap - g0)
                    ps = psump.tile([128, GROUP], dt.float32, tag="ps")
                    for j in range((gw + CHUNK - 1) // CHUNK):
                        c0 = g0 + j * CHUNK
                        w = min(CHUNK, cap - c0)
                        p = 32 * j
                        nc.tensor.matmul(
                            out=ps[:, j * CHUNK:j * CHUNK + w],
                            lhsT=lhs_sb[p:p + KDIM, tcol],
                            rhs=rhs_sb[p:p + KDIM, c0:c0 + w],
                            start=True, stop=True,
                            tile_position=(p, 0),
                        )
                    dst = (s8_all[:, i * KNN:(i + 1) * KNN] if ngroups == 1
                           else cands[:, g * KNN:(g + 1) * KNN])
                    nc.vector.max(out=dst, in_=ps[:, :gw])
                if ngroups > 1:
                    nc.vector.max(out=s8_all[:, i * KNN:(i + 1) * KNN],
                                  in_=cands[:, :])
                off += cap

            # Epilogue: d = sqrt(max(q2 - s, 0)); zero rows with min dist <= 0.25
            d2 = finp.tile([128, RT * KNN], dt.float32)
            nc.vector.tensor_sub(d2[:, :], q2_sb[:, :], s8_all[:, :])
            nc.vector.tensor_scalar_max(d2[:, :], d2[:, :], 0.0)
            dst = finp.tile([128, RT * KNN], dt.float32)
            nc.scalar.activation(dst[:, :], d2[:, :],
                                 mybir.ActivationFunctionType.Sqrt)
            good = finp.tile([128, RT], dt.float32)
            nc.vector.tensor_scalar(good[:, :], dst[:, 0:RT * KNN:KNN],
                                    OCC_RADIUS, None, mybir.AluOpType.is_gt)
            res = finp.tile([128, RT * KNN], dt.float32)
            nc.vector.tensor_tensor(
                res[:, :].rearrange("p (t j) -> p t j", j=KNN),
                dst[:, :].rearrange("p (t j) -> p t j", j=KNN),
                good[:, :, None].broadcast_to([128, RT, KNN]),
                mybir.AluOpType.mult,
            )
            nc.sync.dma_start(
                out=out_d.ap().rearrange("(t p) j -> p t j", p=128),
                in_=res[:, :].rearrange("p (t j) -> p t j", j=KNN),
            )

    nc.compile()
    _CACHE[key] = nc
    return nc


def _run(to_filter, target_coords, trace=False):
    from concourse import bass_utils

    in_maps, row_query, caps = _prep_pruned(to_filter, target_coords)
    nc = _build_pruned(caps)
    res = bass_utils.run_bass_kernel_spmd(
        nc, in_maps, core_ids=list(range(N_CORES)), trace=trace,
    )
    out = np.empty((N, KNN), np.float32)
    for c in range(N_CORES):
        out[row_query[c]] = res.results[c]["out"]
    return out, res


def kernel(to_filter, target_coords):
    out, _ = _run(to_filter, target_coords)
    return out


# revision 6
# speedup vs baseline: 9.1098x; 1.3969x over previous
"""Trainium2 Bass kernel for GuidedImplicitPointSampler KNN (top-8 + occupancy mask).

Strategy (pruned, exact):
  - Host groups the N=32768 queries into 256 spatial tiles of 128 (k-d median
    splits) and, per tile, builds a provably sufficient candidate subset of the
    M=16384 targets from grid cell COUNTS only (no host distance math):
      * ub8(q): walk cell offsets sorted by worst-case point-to-point distance
        until >= 9 targets are guaranteed; d8(q) <= ub8(q).  Two-level grid
        (coarse 0.30 everywhere, fine 0.06 refine in dense regions).
      * tile candidates: every target within R_t = max_q ub8(q) of the tile's
        bbox (cylinder-trimmed cell ranges; superset by construction).
    The device then computes exact distances + top-8 over the candidates, so
    the result equals brute force (candidates contain each query's true 8-NN
    and its nearest target, which also decides the 0.25 occupancy mask).
  - Tiles are dealt to 8 cores x 32 slots (sorted by size, groups of 8) so the
    SPMD program sees identical slot capacities; blocks are sentinel-padded.
  - Per slot: s[n,m] = 2q.k - |k|^2 on the PE as one K=11 fp16 hi/lo matmul
    (error ~2^-22), chunks of <=512 columns packed 4-wide into PE row groups
    0/32/64/96; top-8 via hardware MAX8 straight out of PSUM.
  - Epilogue: d = sqrt(max(q2 - s, 0)), zero rows whose nearest dist <= 0.25;
    host scatters rows back to the original query order.
"""

import numpy as np

N = 32768
M = 16384
KNN = 8
OCC_RADIUS = 0.25
N_CORES = 8
TILE = 128
NTILES = N // TILE            # 256
RT = NTILES // N_CORES        # 32 slots per core
CHUNK = 512                   # matmul moving free dim (one PSUM bank)
PACK = 4                      # concurrent matmuls in PE row-groups
GROUP = CHUNK * PACK          # 2048 target cols per PSUM tile
KDIM = 11
KSAFE = 9
SENTINEL = 60.0

_CACHE = {}


# ---------------------------------------------------------------------------
# Host-side pruning plan (grid counting only, no host distance computations)
# ---------------------------------------------------------------------------

def _cell_counts(pts, lo, h, n):
    ci = np.clip(((pts - lo) / h).astype(np.int64), 0, n - 1)
    cnt = np.zeros((n, n, n), np.int32)
    np.add.at(cnt, (ci[:, 0], ci[:, 1], ci[:, 2]), 1)
    return ci, cnt


def _sorted_offsets(max_cells):
    r = np.arange(-max_cells, max_cells + 1)
    X, Y, Z = np.meshgrid(r, r, r, indexing="ij")
    off = np.stack([X.ravel(), Y.ravel(), Z.ravel()], 1)
    wd = np.sqrt(((np.abs(off) + 1) ** 2).sum(1).astype(np.float64))
    o = np.argsort(wd, kind="stable")
    return off[o], wd[o]


def _walk_ub(cells, cnt, n, offsets, wdist, h, ksafe, chunk=512):
    """Per cell row: smallest wdist*h whose offset-prefix covers >= ksafe targets."""
    U = len(cells)
    ub = np.full(U, np.inf)
    acc = np.zeros(U, np.int64)
    alive = np.arange(U)
    for s in range(0, len(offsets), chunk):
        if len(alive) == 0:
            break
        offs = offsets[s:s + chunk]
        cc = cells[alive][:, None, :] + offs[None, :, :]
        ok = ((cc >= 0) & (cc < n)).all(2)
        cc = np.clip(cc, 0, n - 1)
        counts = cnt[cc[..., 0], cc[..., 1], cc[..., 2]] * ok
        ccum = counts.cumsum(1) + acc[alive][:, None]
        crossed = ccum >= ksafe
        hit = crossed.any(1)
        first = np.argmax(crossed, 1)
        hit_rows = alive[hit]
        ub[hit_rows] = wdist[s + first[hit]] * h
        acc[alive] = ccum[:, -1]
        alive = alive[~hit]
    return ub


def _kd_tiles(q, leaf=TILE):
    out = []

    def rec(ids):
        if len(ids) <= leaf:
            out.append(ids)
            return
        pts = q[ids]
        d = np.argmax(pts.max(0) - pts.min(0))
        half = ((len(ids) // 2) // leaf) * leaf
        o = np.argsort(pts[:, d], kind="stable")
        rec(ids[o[:half]])
        rec(ids[o[half:]])

    rec(np.arange(len(q)))
    return np.concatenate(out)


def _build_plan(q, k, hc=0.30, hf=0.06, hg=0.15, refine_thr=0.9, safety=1.02):
    lo = float(min(q.min(), k.min())) - 1e-4
    hi = float(max(q.max(), k.max())) + 1e-4

    # per-query upper bound on the 8-NN distance (coarse, then fine refine)
    nc_ = int(np.ceil((hi - lo) / hc))
    qic = np.clip(((q - lo) / hc).astype(np.int64), 0, nc_ - 1)
    _, cntc = _cell_counts(k, lo, hc, nc_)
    cells_u, inv = np.unique(qic, axis=0, return_inverse=True)
    offc, wdc = _sorted_offsets(nc_)
    ub = _walk_ub(cells_u, cntc, nc_, offc, wdc, hc, KSAFE)[inv]
    assert np.isfinite(ub).all()

    nf = int(np.ceil((hi - lo) / hf))
    qif = np.clip(((q - lo) / hf).astype(np.int64), 0, nf - 1)
    _, cntf = _cell_counts(k, lo, hf, nf)
    ref = ub <= refine_thr
    if ref.any():
        cells_f, invf = np.unique(qif[ref], axis=0, return_inverse=True)
        offf, wdf = _sorted_offsets(int(np.ceil(refine_thr / hf)) + 1)
        ubf = _walk_ub(cells_f, cntf, nf, offf, wdf, hf, KSAFE)[invf]
        idx = np.nonzero(ref)[0]
        better = ubf < ub[ref]
        ub[idx[better]] = ubf[better]
    ub *= safety

    perm = _kd_tiles(q)

    # gather CSR over the gather grid
    ng = int(np.ceil((hi - lo) / hg))
    kig = np.clip(((k - lo) / hg).astype(np.int64), 0, ng - 1)
    kcell = (kig[:, 0] * ng + kig[:, 1]) * ng + kig[:, 2]
    korder = np.argsort(kcell, kind="stable")
    kcs = kcell[korder]
    starts = np.searchsorted(kcs, np.arange(ng * ng * ng))
    ends = np.searchsorted(kcs, np.arange(ng * ng * ng), side="right")

    cand_lists = []
    for t in range(NTILES):
        qs = perm[t * TILE:(t + 1) * TILE]
        R = float(ub[qs].max())
        R2 = R * R
        blo, bhi = q[qs].min(0), q[qs].max(0)
        a = np.maximum(((blo - R - lo) / hg).astype(np.int64), 0)
        b = np.minimum(((bhi + R - lo) / hg).astype(np.int64), ng - 1)
        parts = []
        for ix in range(a[0], b[0] + 1):
            cx0, cx1 = lo + ix * hg, lo + (ix + 1) * hg
            dx = max(blo[0] - cx1, cx0 - bhi[0], 0.0)
            if dx * dx > R2:
                continue
            for iy in range(a[1], b[1] + 1):
                cy0, cy1 = lo + iy * hg, lo + (iy + 1) * hg
                dy = max(blo[1] - cy1, cy0 - bhi[1], 0.0)
                dxy2 = dx * dx + dy * dy
                if dxy2 > R2:
                    continue
                zh = float(np.sqrt(R2 - dxy2))
                z0 = max(int((blo[2] - zh - lo) / hg), 0)
                z1 = min(int((bhi[2] + zh - lo) / hg), ng - 1)
                base = (ix * ng + iy) * ng
                s, e = starts[base + z0], ends[base + z1]
                if e > s:
                    parts.append(korder[s:e])
        cand_lists.append(np.concatenate(parts) if parts
                          else np.empty(0, np.int64))

    # deal tiles to cores/slots: sort by size asc (small slots first for fast
    # pipeline start), slot i <- tiles [8i, 8i+8)
    sizes = np.array([len(c) for c in cand_lists])
    order = np.argsort(sizes, kind="stable")
    tile_of = order.reshape(RT, N_CORES)            # [slot, core]
    caps = np.empty(RT, np.int64)
    for i in range(RT):
        caps[i] = max(int(np.ceil(sizes[tile_of[i]].max() / 64.0)) * 64, 64)
    return perm, cand_lists, tile_of, caps


def _f16_split(x):
    h = x.astype(np.float16)
    l = (x - h.astype(np.float32)).astype(np.float16)
    return h, l


def _rhs_block(kpts):
    """[11, C] fp16 block: rows = [(2k)h x3, (2k)h x3, (2k)l x3, -|k|2h, -|k|2l]."""
    k2 = (kpts * kpts).sum(1, dtype=np.float32)
    kh, kl = _f16_split(2.0 * kpts.T)
    k2h, k2l = _f16_split(k2)
    blk = np.empty((KDIM, len(kpts)), np.float16)
    blk[0:3] = kh
    blk[3:6] = kh
    blk[6:9] = kl
    blk[9] = -k2h
    blk[10] = -k2l
    return blk


def _prep_pruned(to_filter, target_coords):
    q = np.ascontiguousarray(np.asarray(to_filter, np.float32)[:, :3])
    k = np.ascontiguousarray(np.asarray(target_coords, np.float32)[:, :3])
    perm, cand_lists, tile_of, caps = _build_plan(q, k)
    capsum = int(caps.sum())
    offs = np.concatenate([[0], np.cumsum(caps)]).astype(np.int64)

    sent = np.full(3, SENTINEL, np.float32)
    in_maps = []
    row_query = np.empty((N_CORES, RT * TILE), np.int64)
    for c in range(N_CORES):
        qsel = np.empty(RT * TILE, np.int64)
        rhs_all = np.empty((KDIM, capsum), np.float16)
        for i in range(RT):
            t = tile_of[i, c]
            qsel[i * TILE:(i + 1) * TILE] = perm[t * TILE:(t + 1) * TILE]
            cand = cand_lists[t]
            cap = int(caps[i])
            kp = np.empty((cap, 3), np.float32)
            kp[:len(cand)] = k[cand]
            kp[len(cand):] = sent
            rhs_all[:, offs[i]:offs[i + 1]] = _rhs_block(kp)
        row_query[c] = qsel
        qc = q[qsel]
        q2 = (qc * qc).sum(1, dtype=np.float32)
        qh, ql = _f16_split(qc.T)
        lhsT = np.empty((KDIM, RT * TILE), np.float16)
        lhsT[0:3] = qh
        lhsT[3:6] = ql
        lhsT[6:9] = qh
        lhsT[9] = 1.0
        lhsT[10] = 1.0
        q2c = q2.reshape(RT, TILE).T                       # [128, RT]
        in_maps.append({
            "lhsT": np.ascontiguousarray(lhsT),
            "rhs_all": np.ascontiguousarray(rhs_all),
            "q2rep": np.ascontiguousarray(np.repeat(q2c, KNN, axis=1)),
        })
    return in_maps, row_query, tuple(int(x) for x in caps)


def _build_pruned(caps):
    key = ("pruned", caps)
    if key in _CACHE:
        return _CACHE[key]
    from concourse import bacc, tile, mybir

    dt = mybir.dt
    capsum = sum(caps)
    npc = RT * TILE
    nc = bacc.Bacc("TRN2", target_bir_lowering=False, debug=False,
                   num_devices=N_CORES)

    lhsT_d = nc.dram_tensor("lhsT", [KDIM, npc], dt.float16, kind="ExternalInput")
    rhs_d = nc.dram_tensor("rhs_all", [KDIM, capsum], dt.float16,
                           kind="ExternalInput")
    q2_d = nc.dram_tensor("q2rep", [128, RT * KNN], dt.float32,
                          kind="ExternalInput")
    out_d = nc.dram_tensor("out", [npc, KNN], dt.float32, kind="ExternalOutput")

    with tile.TileContext(nc) as tc:
        with (
            tc.tile_pool(name="const", bufs=1) as constp,
            tc.tile_pool(name="rhs", bufs=4) as rhsp,
            tc.tile_pool(name="psum", bufs=2, space="PSUM") as psump,
            tc.tile_pool(name="cand", bufs=2) as candp,
            tc.tile_pool(name="fin", bufs=1) as finp,
        ):
            q2_sb = constp.tile([128, RT * KNN], dt.float32)
            nc.sync.dma_start(out=q2_sb[:, :], in_=q2_d[:, :])
            lhs_sb = constp.tile([KDIM, npc], dt.float16)
            nc.sync.dma_start(out=lhs_sb[:, :], in_=lhsT_d[:, :])

            s8_all = finp.tile([128, RT * KNN], dt.float32)
            off = 0
            ndma = 0
            for i in range(RT):
                cap = caps[i]
                ngroups = (cap + GROUP - 1) // GROUP
                cands = None
                if ngroups > 1:
                    cands = candp.tile([128, ngroups * KNN], dt.float32,
                                       tag="cands")
                tcol = slice(i * TILE, (i + 1) * TILE)
                for g in range(ngroups):
                    g0 = g * GROUP
                    gw = min(GROUP, cap - g0)
                    rhs_sb = rhsp.tile([KDIM, GROUP], dt.float16, tag="rhs")
                    # alternate the two HWDGE rings (sync / scalar)
                    eng = nc.sync if ndma % 2 == 0 else nc.scalar
                    ndma += 1
                    eng.dma_start(out=rhs_sb[:, :gw],
                                  in_=rhs_d[:, off + g0:off + g0 + gw])
                    ps = psump.tile([128, GROUP], dt.float32, tag="ps")
                    for j in range((gw + CHUNK - 1) // CHUNK):
                        c0 = j * CHUNK
                        w = min(CHUNK, gw - c0)
                        nc.tensor.matmul(
                            out=ps[:, c0:c0 + w],
                            lhsT=lhs_sb[:, tcol],
                            rhs=rhs_sb[:, c0:c0 + w],
                            start=True, stop=True,
                            tile_position=(0, 0),
                        )
                    dst = (s8_all[:, i * KNN:(i + 1) * KNN] if ngroups == 1
                           else cands[:, g * KNN:(g + 1) * KNN])
                    nc.vector.max(out=dst, in_=ps[:, :gw])
                if ngroups > 1:
                    nc.vector.max(out=s8_all[:, i * KNN:(i + 1) * KNN],
                                  in_=cands[:, :])
                off += cap

            # Epilogue: d = sqrt(max(q2 - s, 0)); zero rows with min dist <= 0.25
            d2 = finp.tile([128, RT * KNN], dt.float32)
            nc.vector.tensor_sub(d2[:, :], q2_sb[:, :], s8_all[:, :])
            nc.vector.tensor_scalar_max(d2[:, :], d2[:, :], 0.0)
            dst = finp.tile([128, RT * KNN], dt.float32)
            nc.scalar.activation(dst[:, :], d2[:, :],
                                 mybir.ActivationFunctionType.Sqrt)
            good = finp.tile([128, RT], dt.float32)
            nc.vector.tensor_scalar(good[:, :], dst[:, 0:RT * KNN:KNN],
                                    OCC_RADIUS, None, mybir.AluOpType.is_gt)
            res = finp.tile([128, RT * KNN], dt.float32)
            nc.vector.tensor_tensor(
                res[:, :].rearrange("p (t j) -> p t j", j=KNN),
                dst[:, :].rearrange("p (t j) -> p t j", j=KNN),
                good[:, :, None].broadcast_to([128, RT, KNN]),
                mybir.AluOpType.mult,
            )
            nc.sync.dma_start(
                out=out_d.ap().rearrange("(t p) j -> p t j", p=128),
                in_=res[:, :].rearrange("p (t j) -> p t j", j=KNN),
            )

    nc.compile()
    _CACHE[key] = nc
    return nc


def _run(to_filter, target_coords, trace=False):
    from concourse import bass_utils

    in_maps, row_query, caps = _prep_pruned(to_filter, target_coords)
    nc = _build_pruned(caps)
    res = bass_utils.run_bass_kernel_spmd(
        nc, in_maps, core_ids=list(range(N_CORES)), trace=trace,
    )
    out = np.empty((N, KNN), np.float32)
    for c in range(N_CORES):
        out[row_query[c]] = res.results[c]["out"]
    return out, res


def kernel(to_filter, target_coords):
    out, _ = _run(to_filter, target_coords)
    return out


# revision 12
# speedup vs baseline: 9.8945x; 1.0861x over previous
"""Trainium2 Bass kernel for GuidedImplicitPointSampler KNN (top-8 + occupancy mask).

Strategy (pruned, exact):
  - Host groups the N=32768 queries into 256 spatial tiles of 128 (k-d median
    splits) and, per tile, builds a provably sufficient candidate subset of the
    M=16384 targets from grid cell COUNTS only (no host distance math):
      * ub8(q): walk cell offsets sorted by worst-case point-to-point distance
        until >= 9 targets are guaranteed; d8(q) <= ub8(q).  Two-level grid
        (coarse 0.30 everywhere, fine 0.06 refine in dense regions).
      * tile candidates: every target within R_t = max_q ub8(q) of the tile's
        bbox (cylinder-trimmed cell ranges; superset by construction).
    The device then computes exact distances + top-8 over the candidates, so
    the result equals brute force (candidates contain each query's true 8-NN
    and its nearest target, which also decides the 0.25 occupancy mask).
  - Tiles are dealt to 8 cores x 32 slots (sorted by size, groups of 8) so the
    SPMD program sees identical slot capacities; blocks are sentinel-padded.
  - Per slot: s[n,m] = 2q.k - |k|^2 on the PE as one K=11 fp16 hi/lo matmul
    (error ~2^-22), chunks of <=512 columns packed 4-wide into PE row groups
    0/32/64/96; top-8 via hardware MAX8 straight out of PSUM.
  - Epilogue: d = sqrt(max(q2 - s, 0)), zero rows whose nearest dist <= 0.25;
    host scatters rows back to the original query order.
"""

import numpy as np

N = 32768
M = 16384
KNN = 8
OCC_RADIUS = 0.25
N_CORES = 8
TILE = 128
NTILES = N // TILE            # 256
RT = NTILES // N_CORES        # 32 slots per core
CHUNK = 512                   # matmul moving free dim (one PSUM bank)
PACK = 4                      # concurrent matmuls in PE row-groups
GROUP = CHUNK * PACK          # 2048 target cols per PSUM tile
KDIM = 11
KSAFE = 9
SENTINEL = 60.0

_CACHE = {}


# ---------------------------------------------------------------------------
# Host-side pruning plan (grid counting only, no host distance computations)
# ---------------------------------------------------------------------------

def _cell_counts(pts, lo, h, n):
    ci = np.clip(((pts - lo) / h).astype(np.int64), 0, n - 1)
    cnt = np.zeros((n, n, n), np.int32)
    np.add.at(cnt, (ci[:, 0], ci[:, 1], ci[:, 2]), 1)
    return ci, cnt


def _sorted_offsets(max_cells):
    r = np.arange(-max_cells, max_cells + 1)
    X, Y, Z = np.meshgrid(r, r, r, indexing="ij")
    off = np.stack([X.ravel(), Y.ravel(), Z.ravel()], 1)
    wd = np.sqrt(((np.abs(off) + 1) ** 2).sum(1).astype(np.float64))
    o = np.argsort(wd, kind="stable")
    return off[o], wd[o]


def _walk_ub(cells, cnt, n, offsets, wdist, h, ksafe, chunk=512):
    """Per cell row: smallest wdist*h whose offset-prefix covers >= ksafe targets."""
    U = len(cells)
    ub = np.full(U, np.inf)
    acc = np.zeros(U, np.int64)
    alive = np.arange(U)
    for s in range(0, len(offsets), chunk):
        if len(alive) == 0:
            break
        offs = offsets[s:s + chunk]
        cc = cells[alive][:, None, :] + offs[None, :, :]
        ok = ((cc >= 0) & (cc < n)).all(2)
        cc = np.clip(cc, 0, n - 1)
        counts = cnt[cc[..., 0], cc[..., 1], cc[..., 2]] * ok
        ccum = counts.cumsum(1) + acc[alive][:, None]
        crossed = ccum >= ksafe
        hit = crossed.any(1)
        first = np.argmax(crossed, 1)
        hit_rows = alive[hit]
        ub[hit_rows] = wdist[s + first[hit]] * h
        acc[alive] = ccum[:, -1]
        alive = alive[~hit]
    return ub


def _kd_tiles(q, leaf=TILE):
    out = []

    def rec(ids):
        if len(ids) <= leaf:
            out.append(ids)
            return
        pts = q[ids]
        d = np.argmax(pts.max(0) - pts.min(0))
        half = ((len(ids) // 2) // leaf) * leaf
        o = np.argsort(pts[:, d], kind="stable")
        rec(ids[o[:half]])
        rec(ids[o[half:]])

    rec(np.arange(len(q)))
    return np.concatenate(out)


def _build_plan(q, k, hc=0.30, hm=0.15, hf=0.06, hg=0.15,
                refine_thr_m=3.0, refine_thr=0.9, safety=1.02):
    lo = float(min(q.min(), k.min())) - 1e-4
    hi = float(max(q.max(), k.max())) + 1e-4

    # per-query upper bound on the 8-NN distance: coarse everywhere, then
    # medium / fine refinement where the bound is already small
    nc_ = int(np.ceil((hi - lo) / hc))
    qic = np.clip(((q - lo) / hc).astype(np.int64), 0, nc_ - 1)
    _, cntc = _cell_counts(k, lo, hc, nc_)
    cells_u, inv = np.unique(qic, axis=0, return_inverse=True)
    offc, wdc = _sorted_offsets(nc_)
    ub = _walk_ub(cells_u, cntc, nc_, offc, wdc, hc, KSAFE)[inv]
    assert np.isfinite(ub).all()

    for h_r, thr in ((hm, refine_thr_m), (hf, refine_thr)):
        n_r = int(np.ceil((hi - lo) / h_r))
        qir = np.clip(((q - lo) / h_r).astype(np.int64), 0, n_r - 1)
        _, cnt_r = _cell_counts(k, lo, h_r, n_r)
        ref = ub <= thr
        if not ref.any():
            continue
        cells_r, invr = np.unique(qir[ref], axis=0, return_inverse=True)
        off_r, wd_r = _sorted_offsets(int(np.ceil(thr / h_r)) + 1)
        ubr = _walk_ub(cells_r, cnt_r, n_r, off_r, wd_r, h_r, KSAFE)[invr]
        idx = np.nonzero(ref)[0]
        better = ubr < ub[ref]
        ub[idx[better]] = ubr[better]
    ub *= safety

    perm = _kd_tiles(q)

    # gather CSR over the gather grid
    ng = int(np.ceil((hi - lo) / hg))
    kig = np.clip(((k - lo) / hg).astype(np.int64), 0, ng - 1)
    kcell = (kig[:, 0] * ng + kig[:, 1]) * ng + kig[:, 2]
    korder = np.argsort(kcell, kind="stable")
    kcs = kcell[korder]
    starts = np.searchsorted(kcs, np.arange(ng * ng * ng))
    ends = np.searchsorted(kcs, np.arange(ng * ng * ng), side="right")

    cand_lists = []
    for t in range(NTILES):
        qs = perm[t * TILE:(t + 1) * TILE]
        R = float(ub[qs].max())
        R2 = R * R
        blo, bhi = q[qs].min(0), q[qs].max(0)
        a = np.maximum(((blo - R - lo) / hg).astype(np.int64), 0)
        b = np.minimum(((bhi + R - lo) / hg).astype(np.int64), ng - 1)
        parts = []
        for ix in range(a[0], b[0] + 1):
            cx0, cx1 = lo + ix * hg, lo + (ix + 1) * hg
            dx = max(blo[0] - cx1, cx0 - bhi[0], 0.0)
            if dx * dx > R2:
                continue
            for iy in range(a[1], b[1] + 1):
                cy0, cy1 = lo + iy * hg, lo + (iy + 1) * hg
                dy = max(blo[1] - cy1, cy0 - bhi[1], 0.0)
                dxy2 = dx * dx + dy * dy
                if dxy2 > R2:
                    continue
                zh = float(np.sqrt(R2 - dxy2))
                z0 = max(int((blo[2] - zh - lo) / hg), 0)
                z1 = min(int((bhi[2] + zh - lo) / hg), ng - 1)
                base = (ix * ng + iy) * ng
                s, e = starts[base + z0], ends[base + z1]
                if e > s:
                    parts.append(korder[s:e])
        cand_lists.append(np.concatenate(parts) if parts
                          else np.empty(0, np.int64))

    # deal tiles to cores/slots: sort by size asc (small slots first for fast
    # pipeline start), slot i <- tiles [8i, 8i+8)
    sizes = np.array([len(c) for c in cand_lists])
    order = np.argsort(sizes, kind="stable")
    tile_of = order.reshape(RT, N_CORES)            # [slot, core]
    caps = np.empty(RT, np.int64)
    for i in range(RT):
        caps[i] = max(int(np.ceil(sizes[tile_of[i]].max() / 64.0)) * 64, 64)
    return perm, cand_lists, tile_of, caps


def _f16_split(x):
    h = x.astype(np.float16)
    l = (x - h.astype(np.float32)).astype(np.float16)
    return h, l


def _rhs_block(kpts):
    """[11, C] fp16 block: rows = [(2k)h x3, (2k)h x3, (2k)l x3, -|k|2h, -|k|2l]."""
    k2 = (kpts * kpts).sum(1, dtype=np.float32)
    kh, kl = _f16_split(2.0 * kpts.T)
    k2h, k2l = _f16_split(k2)
    blk = np.empty((KDIM, len(kpts)), np.float16)
    blk[0:3] = kh
    blk[3:6] = kh
    blk[6:9] = kl
    blk[9] = -k2h
    blk[10] = -k2l
    return blk


def _prep_pruned(to_filter, target_coords):
    q = np.ascontiguousarray(np.asarray(to_filter, np.float32)[:, :3])
    k = np.ascontiguousarray(np.asarray(target_coords, np.float32)[:, :3])
    perm, cand_lists, tile_of, caps = _build_plan(q, k)
    capsum = int(caps.sum())
    offs = np.concatenate([[0], np.cumsum(caps)]).astype(np.int64)

    sent = np.full(3, SENTINEL, np.float32)
    in_maps = []
    row_query = np.empty((N_CORES, RT * TILE), np.int64)
    for c in range(N_CORES):
        qsel = np.empty(RT * TILE, np.int64)
        rhs_all = np.empty((KDIM, capsum), np.float16)
        for i in range(RT):
            t = tile_of[i, c]
            qsel[i * TILE:(i + 1) * TILE] = perm[t * TILE:(t + 1) * TILE]
            cand = cand_lists[t]
            cap = int(caps[i])
            kp = np.empty((cap, 3), np.float32)
            kp[:len(cand)] = k[cand]
            kp[len(cand):] = sent
            blk = _rhs_block(kp)
            # per-group column swizzle: [even chunks | odd chunks]
            for g0 in range(0, cap, GROUP):
                gw = min(GROUP, cap - g0)
                cols = np.arange(gw)
                sel = (cols // CHUNK) % 2
                order = np.concatenate([cols[sel == 0], cols[sel == 1]])
                rhs_all[:, offs[i] + g0:offs[i] + g0 + gw] = blk[:, g0 + order]
        row_query[c] = qsel
        qc = q[qsel]
        q2 = (qc * qc).sum(1, dtype=np.float32)
        qh, ql = _f16_split(qc.T)
        lhsT = np.empty((KDIM, RT * TILE), np.float16)
        lhsT[0:3] = qh
        lhsT[3:6] = ql
        lhsT[6:9] = qh
        lhsT[9] = 1.0
        lhsT[10] = 1.0
        q2c = q2.reshape(RT, TILE).T                       # [128, RT]
        in_maps.append({
            "lhsT": np.ascontiguousarray(lhsT),
            "rhs_all": np.ascontiguousarray(rhs_all),
            "q2rep": np.ascontiguousarray(np.repeat(q2c, KNN, axis=1)),
        })
    return in_maps, row_query, tuple(int(x) for x in caps)


def _build_pruned(caps):
    key = ("pruned", caps)
    if key in _CACHE:
        return _CACHE[key]
    from concourse import bacc, tile, mybir

    dt = mybir.dt
    capsum = sum(caps)
    npc = RT * TILE
    nc = bacc.Bacc("TRN2", target_bir_lowering=False, debug=False,
                   num_devices=N_CORES)

    lhsT_d = nc.dram_tensor("lhsT", [KDIM, npc], dt.float16, kind="ExternalInput")
    rhs_d = nc.dram_tensor("rhs_all", [KDIM, capsum], dt.float16,
                           kind="ExternalInput")
    q2_d = nc.dram_tensor("q2rep", [128, RT * KNN], dt.float32,
                          kind="ExternalInput")
    out_d = nc.dram_tensor("out", [128, RT * KNN], dt.float32,
                           kind="ExternalOutput")

    with tile.TileContext(nc) as tc:
        with (
            tc.tile_pool(name="const", bufs=1) as constp,
            tc.tile_pool(name="rhs", bufs=6) as rhsp,
            tc.tile_pool(name="psum", bufs=2, space="PSUM") as psump,
            tc.tile_pool(name="cand", bufs=2) as candp,
            tc.tile_pool(name="fin", bufs=1) as finp,
        ):
            lhs_sb = constp.tile([64, npc], dt.float16)
            nc.sync.dma_start(out=lhs_sb[0:KDIM, :], in_=lhsT_d[:, :])
            nc.scalar.dma_start(out=lhs_sb[32:32 + KDIM, :], in_=lhsT_d[:, :])
            q2_sb = constp.tile([128, RT * KNN], dt.float32)
            nc.sync.dma_start(out=q2_sb[:, :], in_=q2_d[:, :])

            s8_all = finp.tile([128, RT * KNN], dt.float32)
            off = 0
            ndma = 0
            for i in range(RT):
                cap = caps[i]
                ngroups = (cap + GROUP - 1) // GROUP
                cands = None
                if ngroups > 1:
                    cands = candp.tile([128, ngroups * KNN], dt.float32,
                                       tag="cands")
                tcol = slice(i * TILE, (i + 1) * TILE)
                for g in range(ngroups):
                    g0 = g * GROUP
                    gw = min(GROUP, cap - g0)
                    # DRAM group block is column-swizzled [even chunks | odd
                    # chunks]; SBUF keeps evens in rows 0-10, odds in 32-42
                    # (PE row-groups 0 / 32 run concurrently), one HWDGE ring
                    # per half.
                    widths = [min(CHUNK, gw - j * CHUNK)
                              for j in range((gw + CHUNK - 1) // CHUNK)]
                    ev_w = sum(w for j, w in enumerate(widths) if j % 2 == 0)
                    od_w = gw - ev_w
                    rhs_sb = rhsp.tile([64, GROUP], dt.float16, tag="rhs")
                    base = off + g0
                    nc.sync.dma_start(out=rhs_sb[0:KDIM, :ev_w],
                                      in_=rhs_d[:, base:base + ev_w])
                    if od_w:
                        nc.scalar.dma_start(
                            out=rhs_sb[32:32 + KDIM, :od_w],
                            in_=rhs_d[:, base + ev_w:base + gw])
                    ps = psump.tile([128, GROUP], dt.float32, tag="ps")
                    for j, w in enumerate(widths):
                        p = 32 * (j % 2)
                        sb0 = (j // 2) * CHUNK
                        nc.tensor.matmul(
                            out=ps[:, j * CHUNK:j * CHUNK + w],
                            lhsT=lhs_sb[p:p + KDIM, tcol],
                            rhs=rhs_sb[p:p + KDIM, sb0:sb0 + w],
                            start=True, stop=True,
                            tile_position=(p, 0),
                        )
                    dst = (s8_all[:, i * KNN:(i + 1) * KNN] if ngroups == 1
                           else cands[:, g * KNN:(g + 1) * KNN])
                    nc.vector.max(out=dst, in_=ps[:, :gw])
                if ngroups > 1:
                    nc.vector.max(out=s8_all[:, i * KNN:(i + 1) * KNN],
                                  in_=cands[:, :])
                off += cap

            # Epilogue: d = sqrt(max(q2 - s, 0)); zero rows with min dist <= 0.25
            d2 = finp.tile([128, RT * KNN], dt.float32)
            nc.vector.tensor_sub(d2[:, :], q2_sb[:, :], s8_all[:, :])
            nc.vector.tensor_scalar_max(d2[:, :], d2[:, :], 0.0)
            dst = finp.tile([128, RT * KNN], dt.float32)
            nc.scalar.activation(dst[:, :], d2[:, :],
                                 mybir.ActivationFunctionType.Sqrt)
            good = finp.tile([128, RT], dt.float32)
            nc.vector.tensor_scalar(good[:, :], dst[:, 0:RT * KNN:KNN],
                                    OCC_RADIUS, None, mybir.AluOpType.is_gt)
            res = finp.tile([128, RT * KNN], dt.float32)
            nc.vector.tensor_tensor(
                res[:, :].rearrange("p (t j) -> p t j", j=KNN),
                dst[:, :].rearrange("p (t j) -> p t j", j=KNN),
                good[:, :, None].broadcast_to([128, RT, KNN]),
                mybir.AluOpType.mult,
            )
            nc.sync.dma_start(out=out_d.ap(), in_=res[:, :])

    nc.compile()
    _CACHE[key] = nc
    return nc


def _run(to_filter, target_coords, trace=False):
    from concourse import bass_utils

    in_maps, row_query, caps = _prep_pruned(to_filter, target_coords)
    nc = _build_pruned(caps)
    res = bass_utils.run_bass_kernel_spmd(
        nc, in_maps, core_ids=list(range(N_CORES)), trace=trace,
    )
    out = np.empty((N, KNN), np.float32)
    for c in range(N_CORES):
        oc = res.results[c]["out"].reshape(128, RT, KNN)
        out[row_query[c]] = oc.transpose(1, 0, 2).reshape(RT * TILE, KNN)
    return out, res


def kernel(to_filter, target_coords):
    out, _ = _run(to_filter, target_coords)
    return out


# revision 13
# speedup vs baseline: 10.2777x; 1.0387x over previous
"""Trainium2 Bass kernel for GuidedImplicitPointSampler KNN (top-8 + occupancy mask).

Strategy (pruned, exact):
  - Host groups the N=32768 queries into 256 spatial tiles of 128 (k-d median
    splits) and, per tile, builds a provably sufficient candidate subset of the
    M=16384 targets from grid cell COUNTS only (no host distance math):
      * ub8(q): walk cell offsets sorted by worst-case point-to-point distance
        until >= 9 targets are guaranteed; d8(q) <= ub8(q).  Two-level grid
        (coarse 0.30 everywhere, fine 0.06 refine in dense regions).
      * tile candidates: every target within R_t = max_q ub8(q) of the tile's
        bbox (cylinder-trimmed cell ranges; superset by construction).
    The device then computes exact distances + top-8 over the candidates, so
    the result equals brute force (candidates contain each query's true 8-NN
    and its nearest target, which also decides the 0.25 occupancy mask).
  - Tiles are dealt to 8 cores x 32 slots (sorted by size, groups of 8) so the
    SPMD program sees identical slot capacities; blocks are sentinel-padded.
  - Per slot: s[n,m] = 2q.k - |k|^2 on the PE as one K=11 fp16 hi/lo matmul
    (error ~2^-22), chunks of <=512 columns packed 4-wide into PE row groups
    0/32/64/96; top-8 via hardware MAX8 straight out of PSUM.
  - Epilogue: d = sqrt(max(q2 - s, 0)), zero rows whose nearest dist <= 0.25;
    host scatters rows back to the original query order.
"""

import numpy as np

N = 32768
M = 16384
KNN = 8
OCC_RADIUS = 0.25
N_CORES = 8
TILE = 128
NTILES = N // TILE            # 256
RT = NTILES // N_CORES        # 32 slots per core
CHUNK = 512                   # matmul moving free dim (one PSUM bank)
PACK = 4                      # concurrent matmuls in PE row-groups
GROUP = CHUNK * PACK          # 2048 target cols per PSUM tile
KDIM = 11
KSAFE = 9
SENTINEL = 60.0

_CACHE = {}


# ---------------------------------------------------------------------------
# Host-side pruning plan (grid counting only, no host distance computations)
# ---------------------------------------------------------------------------

def _cell_counts(pts, lo, h, n):
    ci = np.clip(((pts - lo) / h).astype(np.int64), 0, n - 1)
    cnt = np.zeros((n, n, n), np.int32)
    np.add.at(cnt, (ci[:, 0], ci[:, 1], ci[:, 2]), 1)
    return ci, cnt


def _sorted_offsets(max_cells):
    r = np.arange(-max_cells, max_cells + 1)
    X, Y, Z = np.meshgrid(r, r, r, indexing="ij")
    off = np.stack([X.ravel(), Y.ravel(), Z.ravel()], 1)
    wd = np.sqrt(((np.abs(off) + 1) ** 2).sum(1).astype(np.float64))
    o = np.argsort(wd, kind="stable")
    return off[o], wd[o]


def _walk_ub(cells, cnt, n, offsets, wdist, h, ksafe, chunk=512):
    """Per cell row: smallest wdist*h whose offset-prefix covers >= ksafe targets."""
    U = len(cells)
    ub = np.full(U, np.inf)
    acc = np.zeros(U, np.int64)
    alive = np.arange(U)
    for s in range(0, len(offsets), chunk):
        if len(alive) == 0:
            break
        offs = offsets[s:s + chunk]
        cc = cells[alive][:, None, :] + offs[None, :, :]
        ok = ((cc >= 0) & (cc < n)).all(2)
        cc = np.clip(cc, 0, n - 1)
        counts = cnt[cc[..., 0], cc[..., 1], cc[..., 2]] * ok
        ccum = counts.cumsum(1) + acc[alive][:, None]
        crossed = ccum >= ksafe
        hit = crossed.any(1)
        first = np.argmax(crossed, 1)
        hit_rows = alive[hit]
        ub[hit_rows] = wdist[s + first[hit]] * h
        acc[alive] = ccum[:, -1]
        alive = alive[~hit]
    return ub


def _kd_tiles(q, leaf=TILE):
    out = []

    def rec(ids):
        if len(ids) <= leaf:
            out.append(ids)
            return
        pts = q[ids]
        d = np.argmax(pts.max(0) - pts.min(0))
        half = ((len(ids) // 2) // leaf) * leaf
        o = np.argsort(pts[:, d], kind="stable")
        rec(ids[o[:half]])
        rec(ids[o[half:]])

    rec(np.arange(len(q)))
    return np.concatenate(out)


def _build_plan(q, k, hc=0.30, hm=0.15, hf=0.06, hg=0.15,
                refine_thr_m=3.0, refine_thr=0.9, safety=1.02):
    lo = float(min(q.min(), k.min())) - 1e-4
    hi = float(max(q.max(), k.max())) + 1e-4

    # per-query upper bound on the 8-NN distance: coarse everywhere, then
    # medium / fine refinement where the bound is already small
    nc_ = int(np.ceil((hi - lo) / hc))
    qic = np.clip(((q - lo) / hc).astype(np.int64), 0, nc_ - 1)
    _, cntc = _cell_counts(k, lo, hc, nc_)
    cells_u, inv = np.unique(qic, axis=0, return_inverse=True)
    offc, wdc = _sorted_offsets(nc_)
    ub = _walk_ub(cells_u, cntc, nc_, offc, wdc, hc, KSAFE)[inv]
    assert np.isfinite(ub).all()

    for h_r, thr in ((hm, refine_thr_m), (hf, refine_thr)):
        n_r = int(np.ceil((hi - lo) / h_r))
        qir = np.clip(((q - lo) / h_r).astype(np.int64), 0, n_r - 1)
        _, cnt_r = _cell_counts(k, lo, h_r, n_r)
        ref = ub <= thr
        if not ref.any():
            continue
        cells_r, invr = np.unique(qir[ref], axis=0, return_inverse=True)
        off_r, wd_r = _sorted_offsets(int(np.ceil(thr / h_r)) + 1)
        ubr = _walk_ub(cells_r, cnt_r, n_r, off_r, wd_r, h_r, KSAFE)[invr]
        idx = np.nonzero(ref)[0]
        better = ubr < ub[ref]
        ub[idx[better]] = ubr[better]
    ub *= safety

    perm = _kd_tiles(q)

    # gather CSR over the gather grid
    ng = int(np.ceil((hi - lo) / hg))
    kig = np.clip(((k - lo) / hg).astype(np.int64), 0, ng - 1)
    kcell = (kig[:, 0] * ng + kig[:, 1]) * ng + kig[:, 2]
    korder = np.argsort(kcell, kind="stable")
    kcs = kcell[korder]
    starts = np.searchsorted(kcs, np.arange(ng * ng * ng))
    ends = np.searchsorted(kcs, np.arange(ng * ng * ng), side="right")

    cand_lists = []
    for t in range(NTILES):
        qs = perm[t * TILE:(t + 1) * TILE]
        R = float(ub[qs].max())
        R2 = R * R
        blo, bhi = q[qs].min(0), q[qs].max(0)
        a = np.maximum(((blo - R - lo) / hg).astype(np.int64), 0)
        b = np.minimum(((bhi + R - lo) / hg).astype(np.int64), ng - 1)
        parts = []
        for ix in range(a[0], b[0] + 1):
            cx0, cx1 = lo + ix * hg, lo + (ix + 1) * hg
            dx = max(blo[0] - cx1, cx0 - bhi[0], 0.0)
            if dx * dx > R2:
                continue
            for iy in range(a[1], b[1] + 1):
                cy0, cy1 = lo + iy * hg, lo + (iy + 1) * hg
                dy = max(blo[1] - cy1, cy0 - bhi[1], 0.0)
                dxy2 = dx * dx + dy * dy
                if dxy2 > R2:
                    continue
                zh = float(np.sqrt(R2 - dxy2))
                z0 = max(int((blo[2] - zh - lo) / hg), 0)
                z1 = min(int((bhi[2] + zh - lo) / hg), ng - 1)
                base = (ix * ng + iy) * ng
                s, e = starts[base + z0], ends[base + z1]
                if e > s:
                    parts.append(korder[s:e])
        cand_lists.append(np.concatenate(parts) if parts
                          else np.empty(0, np.int64))

    # deal tiles to cores/slots: sort by size asc (small slots first for fast
    # pipeline start), slot i <- tiles [8i, 8i+8)
    sizes = np.array([len(c) for c in cand_lists])
    order = np.argsort(sizes, kind="stable")
    tile_of = order.reshape(RT, N_CORES)            # [slot, core]
    caps = np.empty(RT, np.int64)
    for i in range(RT):
        caps[i] = max(int(np.ceil(sizes[tile_of[i]].max() / 64.0)) * 64, 64)
    return perm, cand_lists, tile_of, caps


def _f16_split(x):
    h = x.astype(np.float16)
    l = (x - h.astype(np.float32)).astype(np.float16)
    return h, l


def _rhs_block(kpts):
    """[11, C] fp16 block: rows = [(2k)h x3, (2k)h x3, (2k)l x3, -|k|2h, -|k|2l]."""
    k2 = (kpts * kpts).sum(1, dtype=np.float32)
    kh, kl = _f16_split(2.0 * kpts.T)
    k2h, k2l = _f16_split(k2)
    blk = np.empty((KDIM, len(kpts)), np.float16)
    blk[0:3] = kh
    blk[3:6] = kh
    blk[6:9] = kl
    blk[9] = -k2h
    blk[10] = -k2l
    return blk


def _prep_pruned(to_filter, target_coords):
    q = np.ascontiguousarray(np.asarray(to_filter, np.float32)[:, :3])
    k = np.ascontiguousarray(np.asarray(target_coords, np.float32)[:, :3])
    perm, cand_lists, tile_of, caps = _build_plan(q, k)
    capsum = int(caps.sum())
    offs = np.concatenate([[0], np.cumsum(caps)]).astype(np.int64)

    sent = np.full(3, SENTINEL, np.float32)
    in_maps = []
    row_query = np.empty((N_CORES, RT * TILE), np.int64)
    for c in range(N_CORES):
        qsel = np.empty(RT * TILE, np.int64)
        rhs_all = np.empty((KDIM, capsum), np.float16)
        for i in range(RT):
            t = tile_of[i, c]
            qsel[i * TILE:(i + 1) * TILE] = perm[t * TILE:(t + 1) * TILE]
            cand = cand_lists[t]
            cap = int(caps[i])
            kp = np.empty((cap, 3), np.float32)
            kp[:len(cand)] = k[cand]
            kp[len(cand):] = sent
            blk = _rhs_block(kp)
            # per-group column swizzle: [even chunks | odd chunks]
            for g0 in range(0, cap, GROUP):
                gw = min(GROUP, cap - g0)
                cols = np.arange(gw)
                sel = (cols // CHUNK) % 2
                order = np.concatenate([cols[sel == 0], cols[sel == 1]])
                rhs_all[:, offs[i] + g0:offs[i] + g0 + gw] = blk[:, g0 + order]
        row_query[c] = qsel
        qc = q[qsel]
        q2 = (qc * qc).sum(1, dtype=np.float32)
        qh, ql = _f16_split(qc.T)
        lhsT = np.empty((KDIM, RT * TILE), np.float16)
        lhsT[0:3] = qh
        lhsT[3:6] = ql
        lhsT[6:9] = qh
        lhsT[9] = 1.0
        lhsT[10] = 1.0
        q2c = q2.reshape(RT, TILE).T                       # [128, RT]
        in_maps.append({
            "lhsT": np.ascontiguousarray(lhsT),
            "rhs_all": np.ascontiguousarray(rhs_all),
            "q2rep": np.ascontiguousarray(np.repeat(q2c, KNN, axis=1)),
        })
    return in_maps, row_query, tuple(int(x) for x in caps)


def _build_pruned(caps):
    key = ("pruned", caps)
    if key in _CACHE:
        return _CACHE[key]
    from concourse import bacc, tile, mybir

    dt = mybir.dt
    capsum = sum(caps)
    npc = RT * TILE
    nc = bacc.Bacc("TRN2", target_bir_lowering=False, debug=False,
                   num_devices=N_CORES)

    lhsT_d = nc.dram_tensor("lhsT", [KDIM, npc], dt.float16, kind="ExternalInput")
    rhs_d = nc.dram_tensor("rhs_all", [KDIM, capsum], dt.float16,
                           kind="ExternalInput")
    q2_d = nc.dram_tensor("q2rep", [128, RT * KNN], dt.float32,
                          kind="ExternalInput")
    out_d = nc.dram_tensor("out", [128, RT * KNN], dt.float32,
                           kind="ExternalOutput")

    with tile.TileContext(nc) as tc:
        with (
            tc.tile_pool(name="const", bufs=1) as constp,
            tc.tile_pool(name="rhs", bufs=6) as rhsp,
            tc.tile_pool(name="psum", bufs=2, space="PSUM") as psump,
            tc.tile_pool(name="cand", bufs=2) as candp,
            tc.tile_pool(name="fin", bufs=1) as finp,
        ):
            lhs_sb = constp.tile([64, npc], dt.float16)
            nc.sync.dma_start(out=lhs_sb[0:KDIM, :], in_=lhsT_d[:, :])
            nc.scalar.dma_start(out=lhs_sb[32:32 + KDIM, :], in_=lhsT_d[:, :])

            s8_all = finp.tile([128, RT * KNN], dt.float32)
            offs = [0]
            for cap in caps:
                offs.append(offs[-1] + cap)

            # Small slots (cap <= 1024, sorted first) are batched: one DMA
            # covers several consecutive slots' blocks (kills the ~0.8 us
            # per-DMA latency ramp at kernel start).
            BATCH_MAX = 4096
            i = 0
            nbatch = 0
            while i < RT and caps[i] <= 1024:
                j = i
                tot = 0
                while j < RT and caps[j] <= 1024 and tot + caps[j] <= BATCH_MAX:
                    tot += caps[j]
                    j += 1
                rhs_sb = rhsp.tile([KDIM, BATCH_MAX], dt.float16, tag="rhsb")
                eng = nc.sync if nbatch % 2 == 0 else nc.scalar
                nbatch += 1
                eng.dma_start(out=rhs_sb[:, :tot],
                              in_=rhs_d[:, offs[i]:offs[i] + tot])
                sboff = 0
                for s in range(i, j):
                    cap = caps[s]
                    tcol = slice(s * TILE, (s + 1) * TILE)
                    ps = psump.tile([128, GROUP], dt.float32, tag="ps")
                    for c0 in range(0, cap, CHUNK):
                        w = min(CHUNK, cap - c0)
                        nc.tensor.matmul(
                            out=ps[:, c0:c0 + w],
                            lhsT=lhs_sb[0:KDIM, tcol],
                            rhs=rhs_sb[:, sboff + c0:sboff + c0 + w],
                            start=True, stop=True,
                            tile_position=(0, 0),
                        )
                    nc.vector.max(out=s8_all[:, s * KNN:(s + 1) * KNN],
                                  in_=ps[:, :cap])
                    sboff += cap
                i = j

            # Big slots: per-group dual-ring DMAs, chunks alternate PE
            # row-groups 0/32 (DRAM group block column-swizzled
            # [even chunks | odd chunks]).
            for s in range(i, RT):
                cap = caps[s]
                ngroups = (cap + GROUP - 1) // GROUP
                cands = None
                if ngroups > 1:
                    cands = candp.tile([128, ngroups * KNN], dt.float32,
                                       tag="cands")
                tcol = slice(s * TILE, (s + 1) * TILE)
                for g in range(ngroups):
                    g0 = g * GROUP
                    gw = min(GROUP, cap - g0)
                    widths = [min(CHUNK, gw - j * CHUNK)
                              for j in range((gw + CHUNK - 1) // CHUNK)]
                    ev_w = sum(w for j, w in enumerate(widths) if j % 2 == 0)
                    od_w = gw - ev_w
                    rhs_sb = rhsp.tile([64, GROUP], dt.float16, tag="rhs")
                    base = offs[s] + g0
                    nc.sync.dma_start(out=rhs_sb[0:KDIM, :ev_w],
                                      in_=rhs_d[:, base:base + ev_w])
                    if od_w:
                        nc.scalar.dma_start(
                            out=rhs_sb[32:32 + KDIM, :od_w],
                            in_=rhs_d[:, base + ev_w:base + gw])
                    ps = psump.tile([128, GROUP], dt.float32, tag="ps")
                    for j, w in enumerate(widths):
                        p = 32 * (j % 2)
                        sb0 = (j // 2) * CHUNK
                        nc.tensor.matmul(
                            out=ps[:, j * CHUNK:j * CHUNK + w],
                            lhsT=lhs_sb[p:p + KDIM, tcol],
                            rhs=rhs_sb[p:p + KDIM, sb0:sb0 + w],
                            start=True, stop=True,
                            tile_position=(p, 0),
                        )
                    dst = (s8_all[:, s * KNN:(s + 1) * KNN] if ngroups == 1
                           else cands[:, g * KNN:(g + 1) * KNN])
                    nc.vector.max(out=dst, in_=ps[:, :gw])
                if ngroups > 1:
                    nc.vector.max(out=s8_all[:, s * KNN:(s + 1) * KNN],
                                  in_=cands[:, :])

            q2_sb = constp.tile([128, RT * KNN], dt.float32)
            nc.scalar.dma_start(out=q2_sb[:, :], in_=q2_d[:, :])

            # Epilogue: d = sqrt(max(q2 - s, 0)); zero rows with min dist <= 0.25
            d2 = finp.tile([128, RT * KNN], dt.float32)
            nc.vector.tensor_sub(d2[:, :], q2_sb[:, :], s8_all[:, :])
            nc.vector.tensor_scalar_max(d2[:, :], d2[:, :], 0.0)
            dst = finp.tile([128, RT * KNN], dt.float32)
            nc.scalar.activation(dst[:, :], d2[:, :],
                                 mybir.ActivationFunctionType.Sqrt)
            good = finp.tile([128, RT], dt.float32)
            nc.vector.tensor_scalar(good[:, :], dst[:, 0:RT * KNN:KNN],
                                    OCC_RADIUS, None, mybir.AluOpType.is_gt)
            res = finp.tile([128, RT * KNN], dt.float32)
            nc.vector.tensor_tensor(
                res[:, :].rearrange("p (t j) -> p t j", j=KNN),
                dst[:, :].rearrange("p (t j) -> p t j", j=KNN),
                good[:, :, None].broadcast_to([128, RT, KNN]),
                mybir.AluOpType.mult,
            )
            nc.sync.dma_start(out=out_d.ap(), in_=res[:, :])

    nc.compile()
    _CACHE[key] = nc
    return nc


def _run(to_filter, target_coords, trace=False):
    from concourse import bass_utils

    in_maps, row_query, caps = _prep_pruned(to_filter, target_coords)
    nc = _build_pruned(caps)
    res = bass_utils.run_bass_kernel_spmd(
        nc, in_maps, core_ids=list(range(N_CORES)), trace=trace,
    )
    out = np.empty((N, KNN), np.float32)
    for c in range(N_CORES):
        oc = res.results[c]["out"].reshape(128, RT, KNN)
        out[row_query[c]] = oc.transpose(1, 0, 2).reshape(RT * TILE, KNN)
    return out, res


def kernel(to_filter, target_coords):
    out, _ = _run(to_filter, target_coords)
    return out


# revision 20
# speedup vs baseline: 10.4413x; 1.0159x over previous
"""Trainium2 Bass kernel for GuidedImplicitPointSampler KNN (top-8 + occupancy mask).

Strategy (pruned, exact):
  - Host groups the N=32768 queries into 256 spatial tiles of 128 (k-d median
    splits) and, per tile, builds a provably sufficient candidate subset of the
    M=16384 targets from grid cell COUNTS only (no host distance math):
      * ub8(q): walk cell offsets sorted by worst-case point-to-point distance
        until >= 9 targets are guaranteed; d8(q) <= ub8(q).  Two-level grid
        (coarse 0.30 everywhere, fine 0.06 refine in dense regions).
      * tile candidates: every target within R_t = max_q ub8(q) of the tile's
        bbox (cylinder-trimmed cell ranges; superset by construction).
    The device then computes exact distances + top-8 over the candidates, so
    the result equals brute force (candidates contain each query's true 8-NN
    and its nearest target, which also decides the 0.25 occupancy mask).
  - Tiles are dealt to 8 cores x 32 slots (sorted by size, groups of 8) so the
    SPMD program sees identical slot capacities; blocks are sentinel-padded.
  - Per slot: s[n,m] = 2q.k - |k|^2 on the PE as one K=11 fp16 hi/lo matmul
    (error ~2^-22), chunks of <=512 columns packed 4-wide into PE row groups
    0/32/64/96; top-8 via hardware MAX8 straight out of PSUM.
  - Epilogue: d = sqrt(max(q2 - s, 0)), zero rows whose nearest dist <= 0.25;
    host scatters rows back to the original query order.
"""

import numpy as np

N = 32768
M = 16384
KNN = 8
OCC_RADIUS = 0.25
N_CORES = 8
TILE = 128
NTILES = N // TILE            # 256
RT = NTILES // N_CORES        # 32 slots per core
CHUNK = 512                   # matmul moving free dim (one PSUM bank)
PACK = 4                      # concurrent matmuls in PE row-groups
GROUP = CHUNK * PACK          # 2048 target cols per PSUM tile
KDIM = 11
KSAFE = 9
SENTINEL = 60.0

_CACHE = {}


# ---------------------------------------------------------------------------
# Host-side pruning plan (grid counting only, no host distance computations)
# ---------------------------------------------------------------------------

def _cell_counts(pts, lo, h, n):
    ci = np.clip(((pts - lo) / h).astype(np.int64), 0, n - 1)
    cnt = np.zeros((n, n, n), np.int32)
    np.add.at(cnt, (ci[:, 0], ci[:, 1], ci[:, 2]), 1)
    return ci, cnt


def _sorted_offsets(max_cells):
    r = np.arange(-max_cells, max_cells + 1)
    X, Y, Z = np.meshgrid(r, r, r, indexing="ij")
    off = np.stack([X.ravel(), Y.ravel(), Z.ravel()], 1)
    wd = np.sqrt(((np.abs(off) + 1) ** 2).sum(1).astype(np.float64))
    o = np.argsort(wd, kind="stable")
    return off[o], wd[o]


def _walk_ub(cells, cnt, n, offsets, wdist, h, ksafe, chunk=512):
    """Per cell row: smallest wdist*h whose offset-prefix covers >= ksafe targets."""
    U = len(cells)
    ub = np.full(U, np.inf)
    acc = np.zeros(U, np.int64)
    alive = np.arange(U)
    for s in range(0, len(offsets), chunk):
        if len(alive) == 0:
            break
        offs = offsets[s:s + chunk]
        cc = cells[alive][:, None, :] + offs[None, :, :]
        ok = ((cc >= 0) & (cc < n)).all(2)
        cc = np.clip(cc, 0, n - 1)
        counts = cnt[cc[..., 0], cc[..., 1], cc[..., 2]] * ok
        ccum = counts.cumsum(1) + acc[alive][:, None]
        crossed = ccum >= ksafe
        hit = crossed.any(1)
        first = np.argmax(crossed, 1)
        hit_rows = alive[hit]
        ub[hit_rows] = wdist[s + first[hit]] * h
        acc[alive] = ccum[:, -1]
        alive = alive[~hit]
    return ub


def _kd_tiles(q, leaf=TILE):
    out = []

    def rec(ids):
        if len(ids) <= leaf:
            out.append(ids)
            return
        pts = q[ids]
        d = np.argmax(pts.max(0) - pts.min(0))
        half = ((len(ids) // 2) // leaf) * leaf
        o = np.argsort(pts[:, d], kind="stable")
        rec(ids[o[:half]])
        rec(ids[o[half:]])

    rec(np.arange(len(q)))
    return np.concatenate(out)


def _build_plan(q, k, hc=0.30, hm=0.15, hf=0.06, hg=0.10,
                refine_thr_m=3.0, refine_thr=0.9, safety=1.02):
    lo = float(min(q.min(), k.min())) - 1e-4
    hi = float(max(q.max(), k.max())) + 1e-4

    # per-query upper bound on the 8-NN distance: coarse everywhere, then
    # medium / fine refinement where the bound is already small
    nc_ = int(np.ceil((hi - lo) / hc))
    qic = np.clip(((q - lo) / hc).astype(np.int64), 0, nc_ - 1)
    _, cntc = _cell_counts(k, lo, hc, nc_)
    cells_u, inv = np.unique(qic, axis=0, return_inverse=True)
    offc, wdc = _sorted_offsets(nc_)
    ub = _walk_ub(cells_u, cntc, nc_, offc, wdc, hc, KSAFE)[inv]
    assert np.isfinite(ub).all()

    for h_r, thr in ((hm, refine_thr_m), (hf, refine_thr)):
        n_r = int(np.ceil((hi - lo) / h_r))
        qir = np.clip(((q - lo) / h_r).astype(np.int64), 0, n_r - 1)
        _, cnt_r = _cell_counts(k, lo, h_r, n_r)
        ref = ub <= thr
        if not ref.any():
            continue
        cells_r, invr = np.unique(qir[ref], axis=0, return_inverse=True)
        off_r, wd_r = _sorted_offsets(int(np.ceil(thr / h_r)) + 1)
        ubr = _walk_ub(cells_r, cnt_r, n_r, off_r, wd_r, h_r, KSAFE)[invr]
        idx = np.nonzero(ref)[0]
        better = ubr < ub[ref]
        ub[idx[better]] = ubr[better]
    ub *= safety

    perm = _kd_tiles(q)

    # gather CSR over the gather grid
    ng = int(np.ceil((hi - lo) / hg))
    kig = np.clip(((k - lo) / hg).astype(np.int64), 0, ng - 1)
    kcell = (kig[:, 0] * ng + kig[:, 1]) * ng + kig[:, 2]
    korder = np.argsort(kcell, kind="stable")
    kcs = kcell[korder]
    starts = np.searchsorted(kcs, np.arange(ng * ng * ng))
    ends = np.searchsorted(kcs, np.arange(ng * ng * ng), side="right")

    def gather(qs):
        R = float(ub[qs].max())
        R2 = R * R
        blo, bhi = q[qs].min(0), q[qs].max(0)
        a = np.maximum(((blo - R - lo) / hg).astype(np.int64), 0)
        b = np.minimum(((bhi + R - lo) / hg).astype(np.int64), ng - 1)
        parts = []
        for ix in range(a[0], b[0] + 1):
            cx0, cx1 = lo + ix * hg, lo + (ix + 1) * hg
            dx = max(blo[0] - cx1, cx0 - bhi[0], 0.0)
            if dx * dx > R2:
                continue
            for iy in range(a[1], b[1] + 1):
                cy0, cy1 = lo + iy * hg, lo + (iy + 1) * hg
                dy = max(blo[1] - cy1, cy0 - bhi[1], 0.0)
                dxy2 = dx * dx + dy * dy
                if dxy2 > R2:
                    continue
                zh = float(np.sqrt(R2 - dxy2))
                z0 = max(int((blo[2] - zh - lo) / hg), 0)
                z1 = min(int((bhi[2] + zh - lo) / hg), ng - 1)
                base = (ix * ng + iy) * ng
                s, e = starts[base + z0], ends[base + z1]
                if e > s:
                    parts.append(korder[s:e])
        return (np.concatenate(parts) if parts else np.empty(0, np.int64))

    # adaptive tiles: start from 128-query kd leaves; split a tile while the
    # two halves' candidate sets are sufficiently smaller than the parent's
    tiles = []

    def consider(qs, cand, depth):
        if len(cand) > 2048 and len(qs) >= 64 and depth < 3:
            pts = q[qs]
            dim = np.argmax(pts.max(0) - pts.min(0))
            o = np.argsort(pts[:, dim], kind="stable")
            half = len(qs) // 2
            qa, qb = qs[o[:half]], qs[o[half:]]
            ca, cb = gather(qa), gather(qb)
            if len(ca) + len(cb) + 512 < len(cand):
                consider(qa, ca, depth + 1)
                consider(qb, cb, depth + 1)
                return
        tiles.append((qs, cand))

    for t in range(NTILES):
        qs = perm[t * TILE:(t + 1) * TILE]
        consider(qs, gather(qs), 0)

    # pad tile count to a multiple of N_CORES with empty dummy tiles
    while len(tiles) % N_CORES != 0:
        tiles.append((np.empty(0, np.int64), np.empty(0, np.int64)))

    # deal tiles to cores/slots: sort by size asc (small slots first for fast
    # pipeline start), slot i <- tiles [8i, 8i+8)
    sizes = np.array([len(c) for _, c in tiles])
    order = np.argsort(sizes, kind="stable")
    rt = len(tiles) // N_CORES
    tile_of = order.reshape(rt, N_CORES)            # [slot, core]
    caps = np.empty(rt, np.int64)
    for i in range(rt):
        caps[i] = max(int(np.ceil(sizes[tile_of[i]].max() / 64.0)) * 64, 64)
    return tiles, tile_of, caps


def _f16_split(x):
    h = x.astype(np.float16)
    l = (x - h.astype(np.float32)).astype(np.float16)
    return h, l


def _rhs_block(kpts):
    """[11, C] fp16 block: rows = [(2k)h x3, (2k)h x3, (2k)l x3, -|k|2h, -|k|2l]."""
    k2 = (kpts * kpts).sum(1, dtype=np.float32)
    kh, kl = _f16_split(2.0 * kpts.T)
    k2h, k2l = _f16_split(k2)
    blk = np.empty((KDIM, len(kpts)), np.float16)
    blk[0:3] = kh
    blk[3:6] = kh
    blk[6:9] = kl
    blk[9] = -k2h
    blk[10] = -k2l
    return blk


def _prep_pruned(to_filter, target_coords):
    q = np.ascontiguousarray(np.asarray(to_filter, np.float32)[:, :3])
    k = np.ascontiguousarray(np.asarray(target_coords, np.float32)[:, :3])
    tiles, tile_of, caps = _build_plan(q, k)
    rt = len(caps)
    capsum = int(caps.sum())
    offs = np.concatenate([[0], np.cumsum(caps)]).astype(np.int64)

    sent = np.full(3, SENTINEL, np.float32)
    in_maps = []
    rows_per_core = []
    for c in range(N_CORES):
        qsel = np.zeros((rt, TILE), np.int64)
        rows = []
        rhs_all = np.empty((KDIM, capsum), np.float16)
        for i in range(rt):
            t = tile_of[i, c]
            qs, cand = tiles[t]
            rows.append(qs)
            if len(qs):
                qsel[i, :len(qs)] = qs
                qsel[i, len(qs):] = qs[0]
            cap = int(caps[i])
            kp = np.empty((cap, 3), np.float32)
            kp[:len(cand)] = k[cand]
            kp[len(cand):] = sent
            blk = _rhs_block(kp)
            # per-group column swizzle: [even chunks | odd chunks]
            for g0 in range(0, cap, GROUP):
                gw = min(GROUP, cap - g0)
                cols = np.arange(gw)
                sel = (cols // CHUNK) % 2
                order = np.concatenate([cols[sel == 0], cols[sel == 1]])
                rhs_all[:, offs[i] + g0:offs[i] + g0 + gw] = blk[:, g0 + order]
        rows_per_core.append(rows)
        qc = q[qsel.ravel()]
        q2 = (qc * qc).sum(1, dtype=np.float32)
        qh, ql = _f16_split(qc.T)
        lhsT = np.empty((KDIM, rt * TILE), np.float16)
        lhsT[0:3] = qh
        lhsT[3:6] = ql
        lhsT[6:9] = qh
        lhsT[9] = 1.0
        lhsT[10] = 1.0
        q2c = q2.reshape(rt, TILE).T                       # [128, rt]
        in_maps.append({
            "lhsT": np.ascontiguousarray(lhsT),
            "rhs_all": np.ascontiguousarray(rhs_all),
            "q2rep": np.ascontiguousarray(np.repeat(q2c, KNN, axis=1)),
        })
    return in_maps, rows_per_core, tuple(int(x) for x in caps)


def _build_pruned(caps):
    key = ("pruned", caps)
    if key in _CACHE:
        return _CACHE[key]
    from concourse import bacc, tile, mybir

    dt = mybir.dt
    capsum = sum(caps)
    rt = len(caps)
    npc = rt * TILE
    nc = bacc.Bacc("TRN2", target_bir_lowering=False, debug=False,
                   num_devices=N_CORES)

    lhsT_d = nc.dram_tensor("lhsT", [KDIM, npc], dt.float16, kind="ExternalInput")
    rhs_d = nc.dram_tensor("rhs_all", [KDIM, capsum], dt.float16,
                           kind="ExternalInput")
    q2_d = nc.dram_tensor("q2rep", [128, rt * KNN], dt.float32,
                          kind="ExternalInput")
    out_d = nc.dram_tensor("out", [128, rt * KNN], dt.float32,
                           kind="ExternalOutput")

    with tile.TileContext(nc) as tc:
        with (
            tc.tile_pool(name="const", bufs=1) as constp,
            tc.tile_pool(name="rhs", bufs=6) as rhsp,
            tc.tile_pool(name="psum", bufs=2, space="PSUM") as psump,
            tc.tile_pool(name="cand", bufs=2) as candp,
            tc.tile_pool(name="fin", bufs=1) as finp,
        ):
            # lhs split: a small first piece lets slot 0's matmul start while
            # the rest of the queries stream in
            lhs_sb = constp.tile([64, npc], dt.float16)
            l0 = min(8 * TILE, npc)
            nc.sync.dma_start(out=lhs_sb[0:KDIM, :l0], in_=lhsT_d[:, :l0])
            nc.scalar.dma_start(out=lhs_sb[32:32 + KDIM, :l0],
                                in_=lhsT_d[:, :l0])
            if l0 < npc:
                nc.sync.dma_start(out=lhs_sb[0:KDIM, l0:], in_=lhsT_d[:, l0:])
                nc.scalar.dma_start(out=lhs_sb[32:32 + KDIM, l0:],
                                    in_=lhsT_d[:, l0:])

            s8_all = finp.tile([128, rt * KNN], dt.float32)
            offs = [0]
            for cap in caps:
                offs.append(offs[-1] + cap)

            # Small slots (cap <= 1024, sorted first) are batched: one DMA
            # covers several consecutive slots' blocks (kills the ~0.8 us
            # per-DMA latency ramp at kernel start).
            BATCH_MAX = 4096
            i = 0
            nbatch = 0
            while i < rt and caps[i] <= 1024:
                j = i
                tot = 0
                while j < rt and caps[j] <= 1024 and tot + caps[j] <= BATCH_MAX:
                    tot += caps[j]
                    j += 1
                rhs_sb = rhsp.tile([KDIM, BATCH_MAX], dt.float16, tag="rhsb")
                eng = nc.sync if nbatch % 2 == 0 else nc.scalar
                nbatch += 1
                eng.dma_start(out=rhs_sb[:, :tot],
                              in_=rhs_d[:, offs[i]:offs[i] + tot])
                sboff = 0
                for s in range(i, j):
                    cap = caps[s]
                    tcol = slice(s * TILE, (s + 1) * TILE)
                    ps = psump.tile([128, GROUP], dt.float32, tag="ps")
                    for c0 in range(0, cap, CHUNK):
                        w = min(CHUNK, cap - c0)
                        nc.tensor.matmul(
                            out=ps[:, c0:c0 + w],
                            lhsT=lhs_sb[0:KDIM, tcol],
                            rhs=rhs_sb[:, sboff + c0:sboff + c0 + w],
                            start=True, stop=True,
                            tile_position=(0, 0),
                        )
                    nc.vector.max(out=s8_all[:, s * KNN:(s + 1) * KNN],
                                  in_=ps[:, :cap])
                    sboff += cap
                i = j

            # Big slots: per-group dual-ring DMAs, chunks alternate PE
            # row-groups 0/32 (DRAM group block column-swizzled
            # [even chunks | odd chunks]).
            for s in range(i, rt):
                cap = caps[s]
                ngroups = (cap + GROUP - 1) // GROUP
                cands = None
                if ngroups > 1:
                    cands = candp.tile([128, ngroups * KNN], dt.float32,
                                       tag="cands")
                tcol = slice(s * TILE, (s + 1) * TILE)
                for g in range(ngroups):
                    g0 = g * GROUP
                    gw = min(GROUP, cap - g0)
                    widths = [min(CHUNK, gw - j * CHUNK)
                              for j in range((gw + CHUNK - 1) // CHUNK)]
                    ev_w = sum(w for j, w in enumerate(widths) if j % 2 == 0)
                    od_w = gw - ev_w
                    rhs_sb = rhsp.tile([64, GROUP], dt.float16, tag="rhs")
                    base = offs[s] + g0
                    nc.sync.dma_start(out=rhs_sb[0:KDIM, :ev_w],
                                      in_=rhs_d[:, base:base + ev_w])
                    if od_w:
                        nc.scalar.dma_start(
                            out=rhs_sb[32:32 + KDIM, :od_w],
                            in_=rhs_d[:, base + ev_w:base + gw])
                    ps = psump.tile([128, GROUP], dt.float32, tag="ps")
                    for j, w in enumerate(widths):
                        p = 32 * (j % 2)
                        sb0 = (j // 2) * CHUNK
                        nc.tensor.matmul(
                            out=ps[:, j * CHUNK:j * CHUNK + w],
                            lhsT=lhs_sb[p:p + KDIM, tcol],
                            rhs=rhs_sb[p:p + KDIM, sb0:sb0 + w],
                            start=True, stop=True,
                            tile_position=(p, 0),
                        )
                    dst = (s8_all[:, s * KNN:(s + 1) * KNN] if ngroups == 1
                           else cands[:, g * KNN:(g + 1) * KNN])
                    nc.vector.max(out=dst, in_=ps[:, :gw])
                if ngroups > 1:
                    nc.vector.max(out=s8_all[:, s * KNN:(s + 1) * KNN],
                                  in_=cands[:, :])

            q2_sb = constp.tile([128, rt * KNN], dt.float32)
            nc.scalar.dma_start(out=q2_sb[:, :], in_=q2_d[:, :])

            # Epilogue: d = sqrt(max(q2 - s, 0)); zero rows with min dist <= 0.25
            d2 = finp.tile([128, rt * KNN], dt.float32)
            nc.vector.tensor_sub(d2[:, :], q2_sb[:, :], s8_all[:, :])
            nc.vector.tensor_scalar_max(d2[:, :], d2[:, :], 0.0)
            dst = finp.tile([128, rt * KNN], dt.float32)
            nc.scalar.activation(dst[:, :], d2[:, :],
                                 mybir.ActivationFunctionType.Sqrt)
            good = finp.tile([128, rt], dt.float32)
            nc.vector.tensor_scalar(good[:, :], dst[:, 0:rt * KNN:KNN],
                                    OCC_RADIUS, None, mybir.AluOpType.is_gt)
            res = finp.tile([128, rt * KNN], dt.float32)
            nc.vector.tensor_tensor(
                res[:, :].rearrange("p (t j) -> p t j", j=KNN),
                dst[:, :].rearrange("p (t j) -> p t j", j=KNN),
                good[:, :, None].broadcast_to([128, rt, KNN]),
                mybir.AluOpType.mult,
            )
            nc.sync.dma_start(out=out_d.ap(), in_=res[:, :])

    nc.compile()
    _CACHE[key] = nc
    return nc


def _run(to_filter, target_coords, trace=False):
    from concourse import bass_utils

    in_maps, rows_per_core, caps = _prep_pruned(to_filter, target_coords)
    nc = _build_pruned(caps)
    res = bass_utils.run_bass_kernel_spmd(
        nc, in_maps, core_ids=list(range(N_CORES)), trace=trace,
    )
    rt = len(caps)
    out = np.empty((N, KNN), np.float32)
    for c in range(N_CORES):
        oc = res.results[c]["out"].reshape(128, rt, KNN)
        for i, qs in enumerate(rows_per_core[c]):
            if len(qs):
                out[qs] = oc[:len(qs), i, :]
    return out, res


def kernel(to_filter, target_coords):
    out, _ = _run(to_filter, target_coords)
    return out


# revision 23
# speedup vs baseline: 10.5792x; 1.0132x over previous
"""Trainium2 Bass kernel for GuidedImplicitPointSampler KNN (top-8 + occupancy mask).

Strategy (pruned, exact):
  - Host groups the N=32768 queries into 256 spatial tiles of 128 (k-d median
    splits) and, per tile, builds a provably sufficient candidate subset of the
    M=16384 targets from grid cell COUNTS only (no host distance math):
      * ub8(q): walk cell offsets sorted by worst-case point-to-point distance
        until >= 9 targets are guaranteed; d8(q) <= ub8(q).  Two-level grid
        (coarse 0.30 everywhere, fine 0.06 refine in dense regions).
      * tile candidates: every target within R_t = max_q ub8(q) of the tile's
        bbox (cylinder-trimmed cell ranges; superset by construction).
    The device then computes exact distances + top-8 over the candidates, so
    the result equals brute force (candidates contain each query's true 8-NN
    and its nearest target, which also decides the 0.25 occupancy mask).
  - Tiles are dealt to 8 cores x 32 slots (sorted by size, groups of 8) so the
    SPMD program sees identical slot capacities; blocks are sentinel-padded.
  - Per slot: s[n,m] = 2q.k - |k|^2 on the PE as one K=11 fp16 hi/lo matmul
    (error ~2^-22), chunks of <=512 columns packed 4-wide into PE row groups
    0/32/64/96; top-8 via hardware MAX8 straight out of PSUM.
  - Epilogue: d = sqrt(max(q2 - s, 0)), zero rows whose nearest dist <= 0.25;
    host scatters rows back to the original query order.
"""

import numpy as np

N = 32768
M = 16384
KNN = 8
OCC_RADIUS = 0.25
N_CORES = 8
TILE = 128
NTILES = N // TILE            # 256
RT = NTILES // N_CORES        # 32 slots per core
CHUNK = 512                   # matmul moving free dim (one PSUM bank)
PACK = 4                      # concurrent matmuls in PE row-groups
GROUP = CHUNK * PACK          # 2048 target cols per PSUM tile
KDIM = 11
KSAFE = 9
SENTINEL = 60.0

_CACHE = {}


# ---------------------------------------------------------------------------
# Host-side pruning plan (grid counting only, no host distance computations)
# ---------------------------------------------------------------------------

def _cell_counts(pts, lo, h, n):
    ci = np.clip(((pts - lo) / h).astype(np.int64), 0, n - 1)
    cnt = np.zeros((n, n, n), np.int32)
    np.add.at(cnt, (ci[:, 0], ci[:, 1], ci[:, 2]), 1)
    return ci, cnt


def _sorted_offsets(max_cells):
    r = np.arange(-max_cells, max_cells + 1)
    X, Y, Z = np.meshgrid(r, r, r, indexing="ij")
    off = np.stack([X.ravel(), Y.ravel(), Z.ravel()], 1)
    wd = np.sqrt(((np.abs(off) + 1) ** 2).sum(1).astype(np.float64))
    o = np.argsort(wd, kind="stable")
    return off[o], wd[o]


def _walk_ub(cells, cnt, n, offsets, wdist, h, ksafe, chunk=512):
    """Per cell row: smallest wdist*h whose offset-prefix covers >= ksafe targets."""
    U = len(cells)
    ub = np.full(U, np.inf)
    acc = np.zeros(U, np.int64)
    alive = np.arange(U)
    for s in range(0, len(offsets), chunk):
        if len(alive) == 0:
            break
        offs = offsets[s:s + chunk]
        cc = cells[alive][:, None, :] + offs[None, :, :]
        ok = ((cc >= 0) & (cc < n)).all(2)
        cc = np.clip(cc, 0, n - 1)
        counts = cnt[cc[..., 0], cc[..., 1], cc[..., 2]] * ok
        ccum = counts.cumsum(1) + acc[alive][:, None]
        crossed = ccum >= ksafe
        hit = crossed.any(1)
        first = np.argmax(crossed, 1)
        hit_rows = alive[hit]
        ub[hit_rows] = wdist[s + first[hit]] * h
        acc[alive] = ccum[:, -1]
        alive = alive[~hit]
    return ub


def _kd_tiles(q, leaf=TILE):
    out = []

    def rec(ids):
        if len(ids) <= leaf:
            out.append(ids)
            return
        pts = q[ids]
        d = np.argmax(pts.max(0) - pts.min(0))
        half = ((len(ids) // 2) // leaf) * leaf
        o = np.argsort(pts[:, d], kind="stable")
        rec(ids[o[:half]])
        rec(ids[o[half:]])

    rec(np.arange(len(q)))
    return np.concatenate(out)


def _build_plan(q, k, hc=0.30, hm=0.15, hf=0.06, hg=0.10,
                refine_thr_m=3.0, refine_thr=0.9, safety=1.02):
    lo = float(min(q.min(), k.min())) - 1e-4
    hi = float(max(q.max(), k.max())) + 1e-4

    # per-query upper bound on the 8-NN distance: coarse everywhere, then
    # medium / fine refinement where the bound is already small
    nc_ = int(np.ceil((hi - lo) / hc))
    qic = np.clip(((q - lo) / hc).astype(np.int64), 0, nc_ - 1)
    _, cntc = _cell_counts(k, lo, hc, nc_)
    cells_u, inv = np.unique(qic, axis=0, return_inverse=True)
    offc, wdc = _sorted_offsets(nc_)
    ub = _walk_ub(cells_u, cntc, nc_, offc, wdc, hc, KSAFE)[inv]
    assert np.isfinite(ub).all()

    for h_r, thr in ((hm, refine_thr_m), (hf, refine_thr)):
        n_r = int(np.ceil((hi - lo) / h_r))
        qir = np.clip(((q - lo) / h_r).astype(np.int64), 0, n_r - 1)
        _, cnt_r = _cell_counts(k, lo, h_r, n_r)
        ref = ub <= thr
        if not ref.any():
            continue
        cells_r, invr = np.unique(qir[ref], axis=0, return_inverse=True)
        off_r, wd_r = _sorted_offsets(int(np.ceil(thr / h_r)) + 1)
        ubr = _walk_ub(cells_r, cnt_r, n_r, off_r, wd_r, h_r, KSAFE)[invr]
        idx = np.nonzero(ref)[0]
        better = ubr < ub[ref]
        ub[idx[better]] = ubr[better]
    ub *= safety

    perm = _kd_tiles(q)

    # gather CSR over the gather grid
    ng = int(np.ceil((hi - lo) / hg))
    kig = np.clip(((k - lo) / hg).astype(np.int64), 0, ng - 1)
    kcell = (kig[:, 0] * ng + kig[:, 1]) * ng + kig[:, 2]
    korder = np.argsort(kcell, kind="stable")
    kcs = kcell[korder]
    starts = np.searchsorted(kcs, np.arange(ng * ng * ng))
    ends = np.searchsorted(kcs, np.arange(ng * ng * ng), side="right")

    def gather(qs):
        R = float(ub[qs].max())
        R2 = R * R
        blo, bhi = q[qs].min(0), q[qs].max(0)
        a = np.maximum(((blo - R - lo) / hg).astype(np.int64), 0)
        b = np.minimum(((bhi + R - lo) / hg).astype(np.int64), ng - 1)
        parts = []
        for ix in range(a[0], b[0] + 1):
            cx0, cx1 = lo + ix * hg, lo + (ix + 1) * hg
            dx = max(blo[0] - cx1, cx0 - bhi[0], 0.0)
            if dx * dx > R2:
                continue
            for iy in range(a[1], b[1] + 1):
                cy0, cy1 = lo + iy * hg, lo + (iy + 1) * hg
                dy = max(blo[1] - cy1, cy0 - bhi[1], 0.0)
                dxy2 = dx * dx + dy * dy
                if dxy2 > R2:
                    continue
                zh = float(np.sqrt(R2 - dxy2))
                z0 = max(int((blo[2] - zh - lo) / hg), 0)
                z1 = min(int((bhi[2] + zh - lo) / hg), ng - 1)
                base = (ix * ng + iy) * ng
                s, e = starts[base + z0], ends[base + z1]
                if e > s:
                    parts.append(korder[s:e])
        return (np.concatenate(parts) if parts else np.empty(0, np.int64))

    # adaptive tiles: start from 128-query kd leaves; split a tile while the
    # two halves' candidate sets are sufficiently smaller than the parent's
    tiles = []

    def consider(qs, cand, depth):
        if len(cand) > 2048 and len(qs) >= 64 and depth < 3:
            pts = q[qs]
            dim = np.argmax(pts.max(0) - pts.min(0))
            o = np.argsort(pts[:, dim], kind="stable")
            half = len(qs) // 2
            qa, qb = qs[o[:half]], qs[o[half:]]
            ca, cb = gather(qa), gather(qb)
            if len(ca) + len(cb) + 512 < len(cand):
                consider(qa, ca, depth + 1)
                consider(qb, cb, depth + 1)
                return
        tiles.append((qs, cand))

    for t in range(NTILES):
        qs = perm[t * TILE:(t + 1) * TILE]
        consider(qs, gather(qs), 0)

    # pad tile count to a multiple of N_CORES with empty dummy tiles
    while len(tiles) % N_CORES != 0:
        tiles.append((np.empty(0, np.int64), np.empty(0, np.int64)))

    # deal tiles to cores/slots: sort by size asc (small slots first for fast
    # pipeline start), slot i <- tiles [8i, 8i+8)
    sizes = np.array([len(c) for _, c in tiles])
    order = np.argsort(sizes, kind="stable")
    rt = len(tiles) // N_CORES
    tile_of = order.reshape(rt, N_CORES)            # [slot, core]
    caps = np.empty(rt, np.int64)
    for i in range(rt):
        caps[i] = max(int(np.ceil(sizes[tile_of[i]].max() / 64.0)) * 64, 64)
    return tiles, tile_of, caps


def _f16_split(x):
    h = x.astype(np.float16)
    l = (x - h.astype(np.float32)).astype(np.float16)
    return h, l


def _rhs_block(kpts):
    """[11, C] fp16 block: rows = [(2k)h x3, (2k)h x3, (2k)l x3, -|k|2h, -|k|2l]."""
    k2 = (kpts * kpts).sum(1, dtype=np.float32)
    kh, kl = _f16_split(2.0 * kpts.T)
    k2h, k2l = _f16_split(k2)
    blk = np.empty((KDIM, len(kpts)), np.float16)
    blk[0:3] = kh
    blk[3:6] = kh
    blk[6:9] = kl
    blk[9] = -k2h
    blk[10] = -k2l
    return blk


def _prep_pruned(to_filter, target_coords):
    q = np.ascontiguousarray(np.asarray(to_filter, np.float32)[:, :3])
    k = np.ascontiguousarray(np.asarray(target_coords, np.float32)[:, :3])
    tiles, tile_of, caps = _build_plan(q, k)
    rt = len(caps)
    capsum = int(caps.sum())
    offs = np.concatenate([[0], np.cumsum(caps)]).astype(np.int64)

    sent = np.full(3, SENTINEL, np.float32)
    in_maps = []
    rows_per_core = []
    for c in range(N_CORES):
        qsel = np.zeros((rt, TILE), np.int64)
        rows = []
        rhs_all = np.empty((KDIM, capsum), np.float16)
        for i in range(rt):
            t = tile_of[i, c]
            qs, cand = tiles[t]
            rows.append(qs)
            if len(qs):
                qsel[i, :len(qs)] = qs
                qsel[i, len(qs):] = qs[0]
            cap = int(caps[i])
            kp = np.empty((cap, 3), np.float32)
            kp[:len(cand)] = k[cand]
            kp[len(cand):] = sent
            blk = _rhs_block(kp)
            # per-group column swizzle: [even chunks | odd chunks]
            for g0 in range(0, cap, GROUP):
                gw = min(GROUP, cap - g0)
                cols = np.arange(gw)
                sel = (cols // CHUNK) % 2
                order = np.concatenate([cols[sel == 0], cols[sel == 1]])
                rhs_all[:, offs[i] + g0:offs[i] + g0 + gw] = blk[:, g0 + order]
        rows_per_core.append(rows)
        qc = q[qsel.ravel()]
        q2 = (qc * qc).sum(1, dtype=np.float32)
        qh, ql = _f16_split(qc.T)
        lhsT = np.empty((KDIM, rt * TILE), np.float16)
        lhsT[0:3] = qh
        lhsT[3:6] = ql
        lhsT[6:9] = qh
        lhsT[9] = 1.0
        lhsT[10] = 1.0
        q2c = q2.reshape(rt, TILE).T                       # [128, rt]
        in_maps.append({
            "lhsT": np.ascontiguousarray(lhsT),
            "rhs_all": np.ascontiguousarray(rhs_all),
            "q2rep": np.ascontiguousarray(np.repeat(q2c, KNN, axis=1)),
        })
    return in_maps, rows_per_core, tuple(int(x) for x in caps)


def _build_pruned(caps):
    key = ("pruned", caps)
    if key in _CACHE:
        return _CACHE[key]
    from concourse import bacc, tile, mybir

    dt = mybir.dt
    capsum = sum(caps)
    rt = len(caps)
    npc = rt * TILE
    nc = bacc.Bacc("TRN2", target_bir_lowering=False, debug=False,
                   num_devices=N_CORES)

    lhsT_d = nc.dram_tensor("lhsT", [KDIM, npc], dt.float16, kind="ExternalInput")
    rhs_d = nc.dram_tensor("rhs_all", [KDIM, capsum], dt.float16,
                           kind="ExternalInput")
    q2_d = nc.dram_tensor("q2rep", [128, rt * KNN], dt.float32,
                          kind="ExternalInput")
    out_d = nc.dram_tensor("out", [128, rt * KNN], dt.float32,
                           kind="ExternalOutput")

    with tile.TileContext(nc) as tc:
        with (
            tc.tile_pool(name="const", bufs=1) as constp,
            tc.tile_pool(name="rhs", bufs=6) as rhsp,
            tc.tile_pool(name="psum", bufs=2, space="PSUM") as psump,
            tc.tile_pool(name="cand", bufs=2) as candp,
            tc.tile_pool(name="fin", bufs=1) as finp,
        ):
            # lhs split: a small first piece lets slot 0's matmul start while
            # the rest of the queries stream in
            lhs_sb = constp.tile([64, npc], dt.float16)
            l0 = min(8 * TILE, npc)
            nc.sync.dma_start(out=lhs_sb[0:KDIM, :l0], in_=lhsT_d[:, :l0])
            nc.scalar.dma_start(out=lhs_sb[32:32 + KDIM, :l0],
                                in_=lhsT_d[:, :l0])

            q2_sb = constp.tile([128, rt * KNN], dt.float32)
            s8_all = finp.tile([128, rt * KNN], dt.float32)
            dsq = finp.tile([128, rt * KNN], dt.float32)
            droot = finp.tile([128, rt * KNN], dt.float32)
            good = finp.tile([128, rt], dt.float32)
            res = finp.tile([128, rt * KNN], dt.float32)

            def epilogue(a, b):
                # d = sqrt(max(q2 - s, 0)); zero rows whose min d2 <= OCC^2
                ca, cb = a * KNN, b * KNN
                nc.vector.tensor_sub(dsq[:, ca:cb], q2_sb[:, ca:cb],
                                     s8_all[:, ca:cb])
                nc.vector.tensor_scalar(good[:, a:b], dsq[:, ca:cb:KNN],
                                        OCC_RADIUS * OCC_RADIUS, None,
                                        mybir.AluOpType.is_gt)
                nc.vector.tensor_scalar_max(dsq[:, ca:cb], dsq[:, ca:cb], 0.0)
                nc.scalar.activation(droot[:, ca:cb], dsq[:, ca:cb],
                                     mybir.ActivationFunctionType.Sqrt)
                nc.vector.tensor_tensor(
                    res[:, ca:cb].rearrange("p (t j) -> p t j", j=KNN),
                    droot[:, ca:cb].rearrange("p (t j) -> p t j", j=KNN),
                    good[:, a:b, None].broadcast_to([128, b - a, KNN]),
                    mybir.AluOpType.mult,
                )
                nc.sync.dma_start(out=out_d.ap()[:, ca:cb],
                                  in_=res[:, ca:cb])

            offs = [0]
            for cap in caps:
                offs.append(offs[-1] + cap)

            # Small slots (cap <= 1024, sorted first) are batched: one DMA
            # covers several consecutive slots' blocks (kills the ~0.8 us
            # per-DMA latency ramp at kernel start).
            i = 0
            nbatch = 0
            lhs_rest_sent = False
            while i < rt and caps[i] <= 1024:
                j = i
                tot = 0
                bmax = 1024 if nbatch == 0 else 4096
                while j < rt and caps[j] <= 1024 and tot + caps[j] <= bmax:
                    tot += caps[j]
                    j += 1
                rhs_sb = rhsp.tile([KDIM, 4096], dt.float16, tag="rhsb")
                eng = nc.sync if nbatch % 2 == 0 else nc.scalar
                nbatch += 1
                eng.dma_start(out=rhs_sb[:, :tot],
                              in_=rhs_d[:, offs[i]:offs[i] + tot])
                if nbatch == 2 and not lhs_rest_sent and l0 < npc:
                    # stream the rest of the queries + q2 behind the first
                    # two rhs batches
                    nc.sync.dma_start(out=lhs_sb[0:KDIM, l0:],
                                      in_=lhsT_d[:, l0:])
                    nc.scalar.dma_start(out=lhs_sb[32:32 + KDIM, l0:],
                                        in_=lhsT_d[:, l0:])
                    nc.sync.dma_start(out=q2_sb[:, :], in_=q2_d[:, :])
                    lhs_rest_sent = True
                sboff = 0
                for s in range(i, j):
                    cap = caps[s]
                    tcol = slice(s * TILE, (s + 1) * TILE)
                    ps = psump.tile([128, GROUP], dt.float32, tag="ps")
                    for c0 in range(0, cap, CHUNK):
                        w = min(CHUNK, cap - c0)
                        nc.tensor.matmul(
                            out=ps[:, c0:c0 + w],
                            lhsT=lhs_sb[0:KDIM, tcol],
                            rhs=rhs_sb[:, sboff + c0:sboff + c0 + w],
                            start=True, stop=True,
                            tile_position=(0, 0),
                        )
                    nc.vector.max(out=s8_all[:, s * KNN:(s + 1) * KNN],
                                  in_=ps[:, :cap])
                    sboff += cap
                i = j
            if not lhs_rest_sent and l0 < npc:
                nc.sync.dma_start(out=lhs_sb[0:KDIM, l0:], in_=lhsT_d[:, l0:])
                nc.scalar.dma_start(out=lhs_sb[32:32 + KDIM, l0:],
                                    in_=lhsT_d[:, l0:])
                nc.sync.dma_start(out=q2_sb[:, :], in_=q2_d[:, :])
                lhs_rest_sent = True
            epi_done = i
            epilogue(0, epi_done)

            # Big slots: per-group dual-ring DMAs, chunks alternate PE
            # row-groups 0/32 (DRAM group block column-swizzled
            # [even chunks | odd chunks]).
            for s in range(i, rt):
                cap = caps[s]
                ngroups = (cap + GROUP - 1) // GROUP
                cands = None
                if ngroups > 1:
                    cands = candp.tile([128, ngroups * KNN], dt.float32,
                                       tag="cands")
                tcol = slice(s * TILE, (s + 1) * TILE)
                for g in range(ngroups):
                    g0 = g * GROUP
                    gw = min(GROUP, cap - g0)
                    widths = [min(CHUNK, gw - j * CHUNK)
                              for j in range((gw + CHUNK - 1) // CHUNK)]
                    ev_w = sum(w for j, w in enumerate(widths) if j % 2 == 0)
                    od_w = gw - ev_w
                    rhs_sb = rhsp.tile([64, GROUP], dt.float16, tag="rhs")
                    base = offs[s] + g0
                    nc.sync.dma_start(out=rhs_sb[0:KDIM, :ev_w],
                                      in_=rhs_d[:, base:base + ev_w])
                    if od_w:
                        nc.scalar.dma_start(
                            out=rhs_sb[32:32 + KDIM, :od_w],
                            in_=rhs_d[:, base + ev_w:base + gw])
                    ps = psump.tile([128, GROUP], dt.float32, tag="ps")
                    for j, w in enumerate(widths):
                        p = 32 * (j % 2)
                        sb0 = (j // 2) * CHUNK
                        nc.tensor.matmul(
                            out=ps[:, j * CHUNK:j * CHUNK + w],
                            lhsT=lhs_sb[p:p + KDIM, tcol],
                            rhs=rhs_sb[p:p + KDIM, sb0:sb0 + w],
                            start=True, stop=True,
                            tile_position=(p, 0),
                        )
                    dst = (s8_all[:, s * KNN:(s + 1) * KNN] if ngroups == 1
                           else cands[:, g * KNN:(g + 1) * KNN])
                    nc.vector.max(out=dst, in_=ps[:, :gw])
                if ngroups > 1:
                    nc.vector.max(out=s8_all[:, s * KNN:(s + 1) * KNN],
                                  in_=cands[:, :])
            if epi_done < rt:
                epilogue(epi_done, rt)

    nc.compile()
    _CACHE[key] = nc
    return nc


def _run(to_filter, target_coords, trace=False):
    from concourse import bass_utils

    in_maps, rows_per_core, caps = _prep_pruned(to_filter, target_coords)
    nc = _build_pruned(caps)
    res = bass_utils.run_bass_kernel_spmd(
        nc, in_maps, core_ids=list(range(N_CORES)), trace=trace,
    )
    rt = len(caps)
    out = np.empty((N, KNN), np.float32)
    for c in range(N_CORES):
        oc = res.results[c]["out"].reshape(128, rt, KNN)
        for i, qs in enumerate(rows_per_core[c]):
            if len(qs):
                out[qs] = oc[:len(qs), i, :]
    return out, res


def kernel(to_filter, target_coords):
    out, _ = _run(to_filter, target_coords)
    return out


# revision 24
# speedup vs baseline: 11.1389x; 1.0529x over previous
"""Trainium2 Bass kernel for GuidedImplicitPointSampler KNN (top-8 + occupancy mask).

Strategy (pruned, exact):
  - Host groups the N=32768 queries into 256 spatial tiles of 128 (k-d median
    splits) and, per tile, builds a provably sufficient candidate subset of the
    M=16384 targets from grid cell COUNTS only (no host distance math):
      * ub8(q): walk cell offsets sorted by worst-case point-to-point distance
        until >= 9 targets are guaranteed; d8(q) <= ub8(q).  Two-level grid
        (coarse 0.30 everywhere, fine 0.06 refine in dense regions).
      * tile candidates: every target within R_t = max_q ub8(q) of the tile's
        bbox (cylinder-trimmed cell ranges; superset by construction).
    The device then computes exact distances + top-8 over the candidates, so
    the result equals brute force (candidates contain each query's true 8-NN
    and its nearest target, which also decides the 0.25 occupancy mask).
  - Tiles are dealt to 8 cores x 32 slots (sorted by size, groups of 8) so the
    SPMD program sees identical slot capacities; blocks are sentinel-padded.
  - Per slot: s[n,m] = 2q.k - |k|^2 on the PE as one K=11 fp16 hi/lo matmul
    (error ~2^-22), chunks of <=512 columns packed 4-wide into PE row groups
    0/32/64/96; top-8 via hardware MAX8 straight out of PSUM.
  - Epilogue: d = sqrt(max(q2 - s, 0)), zero rows whose nearest dist <= 0.25;
    host scatters rows back to the original query order.
"""

import numpy as np

N = 32768
M = 16384
KNN = 8
OCC_RADIUS = 0.25
N_CORES = 8
TILE = 128
NTILES = N // TILE            # 256
RT = NTILES // N_CORES        # 32 slots per core
CHUNK = 512                   # matmul moving free dim (one PSUM bank)
PACK = 4                      # concurrent matmuls in PE row-groups
GROUP = CHUNK * PACK          # 2048 target cols per PSUM tile
KDIM = 11
KSAFE = 9
SENTINEL = 60.0

_CACHE = {}


# ---------------------------------------------------------------------------
# Host-side pruning plan (grid counting only, no host distance computations)
# ---------------------------------------------------------------------------

def _cell_counts(pts, lo, h, n):
    ci = np.clip(((pts - lo) / h).astype(np.int64), 0, n - 1)
    cnt = np.zeros((n, n, n), np.int32)
    np.add.at(cnt, (ci[:, 0], ci[:, 1], ci[:, 2]), 1)
    return ci, cnt


def _sorted_offsets(max_cells):
    r = np.arange(-max_cells, max_cells + 1)
    X, Y, Z = np.meshgrid(r, r, r, indexing="ij")
    off = np.stack([X.ravel(), Y.ravel(), Z.ravel()], 1)
    wd = np.sqrt(((np.abs(off) + 1) ** 2).sum(1).astype(np.float64))
    o = np.argsort(wd, kind="stable")
    return off[o], wd[o]


def _walk_ub(cells, cnt, n, offsets, wdist, h, ksafe, chunk=512):
    """Per cell row: smallest wdist*h whose offset-prefix covers >= ksafe targets."""
    U = len(cells)
    ub = np.full(U, np.inf)
    acc = np.zeros(U, np.int64)
    alive = np.arange(U)
    for s in range(0, len(offsets), chunk):
        if len(alive) == 0:
            break
        offs = offsets[s:s + chunk]
        cc = cells[alive][:, None, :] + offs[None, :, :]
        ok = ((cc >= 0) & (cc < n)).all(2)
        cc = np.clip(cc, 0, n - 1)
        counts = cnt[cc[..., 0], cc[..., 1], cc[..., 2]] * ok
        ccum = counts.cumsum(1) + acc[alive][:, None]
        crossed = ccum >= ksafe
        hit = crossed.any(1)
        first = np.argmax(crossed, 1)
        hit_rows = alive[hit]
        ub[hit_rows] = wdist[s + first[hit]] * h
        acc[alive] = ccum[:, -1]
        alive = alive[~hit]
    return ub


def _kd_tiles(q, leaf=TILE):
    out = []

    def rec(ids):
        if len(ids) <= leaf:
            out.append(ids)
            return
        pts = q[ids]
        d = np.argmax(pts.max(0) - pts.min(0))
        half = ((len(ids) // 2) // leaf) * leaf
        o = np.argsort(pts[:, d], kind="stable")
        rec(ids[o[:half]])
        rec(ids[o[half:]])

    rec(np.arange(len(q)))
    return np.concatenate(out)


def _build_plan(q, k, hc=0.30, hm=0.15, hf=0.05, hg=0.08,
                refine_thr_m=3.0, refine_thr=0.9, safety=1.01):
    lo = float(min(q.min(), k.min())) - 1e-4
    hi = float(max(q.max(), k.max())) + 1e-4

    # per-query upper bound on the 8-NN distance: coarse everywhere, then
    # medium / fine refinement where the bound is already small
    nc_ = int(np.ceil((hi - lo) / hc))
    qic = np.clip(((q - lo) / hc).astype(np.int64), 0, nc_ - 1)
    _, cntc = _cell_counts(k, lo, hc, nc_)
    cells_u, inv = np.unique(qic, axis=0, return_inverse=True)
    offc, wdc = _sorted_offsets(nc_)
    ub = _walk_ub(cells_u, cntc, nc_, offc, wdc, hc, KSAFE)[inv]
    assert np.isfinite(ub).all()

    for h_r, thr in ((hm, refine_thr_m), (hf, refine_thr)):
        n_r = int(np.ceil((hi - lo) / h_r))
        qir = np.clip(((q - lo) / h_r).astype(np.int64), 0, n_r - 1)
        _, cnt_r = _cell_counts(k, lo, h_r, n_r)
        ref = ub <= thr
        if not ref.any():
            continue
        cells_r, invr = np.unique(qir[ref], axis=0, return_inverse=True)
        off_r, wd_r = _sorted_offsets(int(np.ceil(thr / h_r)) + 1)
        ubr = _walk_ub(cells_r, cnt_r, n_r, off_r, wd_r, h_r, KSAFE)[invr]
        idx = np.nonzero(ref)[0]
        better = ubr < ub[ref]
        ub[idx[better]] = ubr[better]
    ub *= safety

    perm = _kd_tiles(q)

    # gather CSR over the gather grid
    ng = int(np.ceil((hi - lo) / hg))
    kig = np.clip(((k - lo) / hg).astype(np.int64), 0, ng - 1)
    kcell = (kig[:, 0] * ng + kig[:, 1]) * ng + kig[:, 2]
    korder = np.argsort(kcell, kind="stable")
    kcs = kcell[korder]
    starts = np.searchsorted(kcs, np.arange(ng * ng * ng))
    ends = np.searchsorted(kcs, np.arange(ng * ng * ng), side="right")

    def gather(qs):
        R = float(ub[qs].max())
        R2 = R * R
        blo, bhi = q[qs].min(0), q[qs].max(0)
        a = np.maximum(((blo - R - lo) / hg).astype(np.int64), 0)
        b = np.minimum(((bhi + R - lo) / hg).astype(np.int64), ng - 1)
        parts = []
        for ix in range(a[0], b[0] + 1):
            cx0, cx1 = lo + ix * hg, lo + (ix + 1) * hg
            dx = max(blo[0] - cx1, cx0 - bhi[0], 0.0)
            if dx * dx > R2:
                continue
            for iy in range(a[1], b[1] + 1):
                cy0, cy1 = lo + iy * hg, lo + (iy + 1) * hg
                dy = max(blo[1] - cy1, cy0 - bhi[1], 0.0)
                dxy2 = dx * dx + dy * dy
                if dxy2 > R2:
                    continue
                zh = float(np.sqrt(R2 - dxy2))
                z0 = max(int((blo[2] - zh - lo) / hg), 0)
                z1 = min(int((bhi[2] + zh - lo) / hg), ng - 1)
                base = (ix * ng + iy) * ng
                s, e = starts[base + z0], ends[base + z1]
                if e > s:
                    parts.append(korder[s:e])
        return (np.concatenate(parts) if parts else np.empty(0, np.int64))

    # adaptive tiles: start from 128-query kd leaves; split a tile while the
    # two halves' candidate sets are sufficiently smaller than the parent's
    tiles = []

    def consider(qs, cand, depth):
        if len(cand) > 1024 and len(qs) >= 64 and depth < 4:
            pts = q[qs]
            dim = np.argmax(pts.max(0) - pts.min(0))
            o = np.argsort(pts[:, dim], kind="stable")
            half = len(qs) // 2
            qa, qb = qs[o[:half]], qs[o[half:]]
            ca, cb = gather(qa), gather(qb)
            if len(ca) + len(cb) + 384 < len(cand):
                consider(qa, ca, depth + 1)
                consider(qb, cb, depth + 1)
                return
        tiles.append((qs, cand))

    for t in range(NTILES):
        qs = perm[t * TILE:(t + 1) * TILE]
        consider(qs, gather(qs), 0)

    # pad tile count to a multiple of N_CORES with empty dummy tiles
    while len(tiles) % N_CORES != 0:
        tiles.append((np.empty(0, np.int64), np.empty(0, np.int64)))

    # deal tiles to cores/slots: sort by size asc (small slots first for fast
    # pipeline start), slot i <- tiles [8i, 8i+8)
    sizes = np.array([len(c) for _, c in tiles])
    order = np.argsort(sizes, kind="stable")
    rt = len(tiles) // N_CORES
    tile_of = order.reshape(rt, N_CORES)            # [slot, core]
    caps = np.empty(rt, np.int64)
    for i in range(rt):
        caps[i] = max(int(np.ceil(sizes[tile_of[i]].max() / 64.0)) * 64, 64)
    return tiles, tile_of, caps


def _f16_split(x):
    h = x.astype(np.float16)
    l = (x - h.astype(np.float32)).astype(np.float16)
    return h, l


def _rhs_block(kpts):
    """[11, C] fp16 block: rows = [(2k)h x3, (2k)h x3, (2k)l x3, -|k|2h, -|k|2l]."""
    k2 = (kpts * kpts).sum(1, dtype=np.float32)
    kh, kl = _f16_split(2.0 * kpts.T)
    k2h, k2l = _f16_split(k2)
    blk = np.empty((KDIM, len(kpts)), np.float16)
    blk[0:3] = kh
    blk[3:6] = kh
    blk[6:9] = kl
    blk[9] = -k2h
    blk[10] = -k2l
    return blk


def _prep_pruned(to_filter, target_coords):
    q = np.ascontiguousarray(np.asarray(to_filter, np.float32)[:, :3])
    k = np.ascontiguousarray(np.asarray(target_coords, np.float32)[:, :3])
    tiles, tile_of, caps = _build_plan(q, k)
    rt = len(caps)
    capsum = int(caps.sum())
    offs = np.concatenate([[0], np.cumsum(caps)]).astype(np.int64)

    sent = np.full(3, SENTINEL, np.float32)
    in_maps = []
    rows_per_core = []
    for c in range(N_CORES):
        qsel = np.zeros((rt, TILE), np.int64)
        rows = []
        rhs_all = np.empty((KDIM, capsum), np.float16)
        for i in range(rt):
            t = tile_of[i, c]
            qs, cand = tiles[t]
            rows.append(qs)
            if len(qs):
                qsel[i, :len(qs)] = qs
                qsel[i, len(qs):] = qs[0]
            cap = int(caps[i])
            kp = np.empty((cap, 3), np.float32)
            kp[:len(cand)] = k[cand]
            kp[len(cand):] = sent
            blk = _rhs_block(kp)
            # per-group column swizzle: [even chunks | odd chunks]
            for g0 in range(0, cap, GROUP):
                gw = min(GROUP, cap - g0)
                cols = np.arange(gw)
                sel = (cols // CHUNK) % 2
                order = np.concatenate([cols[sel == 0], cols[sel == 1]])
                rhs_all[:, offs[i] + g0:offs[i] + g0 + gw] = blk[:, g0 + order]
        rows_per_core.append(rows)
        qc = q[qsel.ravel()]
        q2 = (qc * qc).sum(1, dtype=np.float32)
        qh, ql = _f16_split(qc.T)
        lhsT = np.empty((KDIM, rt * TILE), np.float16)
        lhsT[0:3] = qh
        lhsT[3:6] = ql
        lhsT[6:9] = qh
        lhsT[9] = 1.0
        lhsT[10] = 1.0
        q2c = q2.reshape(rt, TILE).T                       # [128, rt]
        in_maps.append({
            "lhsT": np.ascontiguousarray(lhsT),
            "rhs_all": np.ascontiguousarray(rhs_all),
            "q2rep": np.ascontiguousarray(np.repeat(q2c, KNN, axis=1)),
        })
    return in_maps, rows_per_core, tuple(int(x) for x in caps)


def _build_pruned(caps):
    key = ("pruned", caps)
    if key in _CACHE:
        return _CACHE[key]
    from concourse import bacc, tile, mybir

    dt = mybir.dt
    capsum = sum(caps)
    rt = len(caps)
    npc = rt * TILE
    nc = bacc.Bacc("TRN2", target_bir_lowering=False, debug=False,
                   num_devices=N_CORES)

    lhsT_d = nc.dram_tensor("lhsT", [KDIM, npc], dt.float16, kind="ExternalInput")
    rhs_d = nc.dram_tensor("rhs_all", [KDIM, capsum], dt.float16,
                           kind="ExternalInput")
    q2_d = nc.dram_tensor("q2rep", [128, rt * KNN], dt.float32,
                          kind="ExternalInput")
    out_d = nc.dram_tensor("out", [128, rt * KNN], dt.float32,
                           kind="ExternalOutput")

    with tile.TileContext(nc) as tc:
        with (
            tc.tile_pool(name="const", bufs=1) as constp,
            tc.tile_pool(name="rhs", bufs=6) as rhsp,
            tc.tile_pool(name="psum", bufs=2, space="PSUM") as psump,
            tc.tile_pool(name="cand", bufs=2) as candp,
            tc.tile_pool(name="fin", bufs=1) as finp,
        ):
            # lhs split: a small first piece lets slot 0's matmul start while
            # the rest of the queries stream in
            lhs_sb = constp.tile([64, npc], dt.float16)
            l0 = min(8 * TILE, npc)
            nc.sync.dma_start(out=lhs_sb[0:KDIM, :l0], in_=lhsT_d[:, :l0])
            nc.scalar.dma_start(out=lhs_sb[32:32 + KDIM, :l0],
                                in_=lhsT_d[:, :l0])

            # q2 rides the SWDGE path so it never queues behind the rhs
            # stream on either HWDGE ring
            q2_sb = constp.tile([128, rt * KNN], dt.float32)
            nc.gpsimd.dma_start(out=q2_sb[:, :], in_=q2_d[:, :])
            s8_all = finp.tile([128, rt * KNN], dt.float32)
            dsq = finp.tile([128, rt * KNN], dt.float32)
            droot = finp.tile([128, rt * KNN], dt.float32)
            good = finp.tile([128, rt], dt.float32)
            res = finp.tile([128, rt * KNN], dt.float32)

            def epilogue(a, b):
                # d = sqrt(max(q2 - s, 0)); zero rows whose min d2 <= OCC^2
                ca, cb = a * KNN, b * KNN
                nc.vector.tensor_sub(dsq[:, ca:cb], q2_sb[:, ca:cb],
                                     s8_all[:, ca:cb])
                nc.vector.tensor_scalar(good[:, a:b], dsq[:, ca:cb:KNN],
                                        OCC_RADIUS * OCC_RADIUS, None,
                                        mybir.AluOpType.is_gt)
                nc.vector.tensor_scalar_max(dsq[:, ca:cb], dsq[:, ca:cb], 0.0)
                nc.scalar.activation(droot[:, ca:cb], dsq[:, ca:cb],
                                     mybir.ActivationFunctionType.Sqrt)
                nc.vector.tensor_tensor(
                    res[:, ca:cb].rearrange("p (t j) -> p t j", j=KNN),
                    droot[:, ca:cb].rearrange("p (t j) -> p t j", j=KNN),
                    good[:, a:b, None].broadcast_to([128, b - a, KNN]),
                    mybir.AluOpType.mult,
                )
                nc.sync.dma_start(out=out_d.ap()[:, ca:cb],
                                  in_=res[:, ca:cb])

            offs = [0]
            for cap in caps:
                offs.append(offs[-1] + cap)

            # Small slots (cap <= 1024, sorted first) are batched: one DMA
            # covers several consecutive slots' blocks (kills the ~0.8 us
            # per-DMA latency ramp at kernel start).
            i = 0
            nbatch = 0
            lhs_rest_sent = False
            while i < rt and caps[i] <= 1024:
                j = i
                tot = 0
                bmax = 1024 if nbatch == 0 else 4096
                while j < rt and caps[j] <= 1024 and tot + caps[j] <= bmax:
                    tot += caps[j]
                    j += 1
                rhs_sb = rhsp.tile([KDIM, 4096], dt.float16, tag="rhsb")
                eng = nc.sync if nbatch % 2 == 0 else nc.scalar
                nbatch += 1
                eng.dma_start(out=rhs_sb[:, :tot],
                              in_=rhs_d[:, offs[i]:offs[i] + tot])
                if nbatch == 2 and not lhs_rest_sent and l0 < npc:
                    # stream the rest of the queries + q2 behind the first
                    # two rhs batches
                    nc.sync.dma_start(out=lhs_sb[0:KDIM, l0:],
                                      in_=lhsT_d[:, l0:])
                    nc.scalar.dma_start(out=lhs_sb[32:32 + KDIM, l0:],
                                        in_=lhsT_d[:, l0:])
                    lhs_rest_sent = True
                sboff = 0
                for s in range(i, j):
                    cap = caps[s]
                    tcol = slice(s * TILE, (s + 1) * TILE)
                    ps = psump.tile([128, GROUP], dt.float32, tag="ps")
                    for c0 in range(0, cap, CHUNK):
                        w = min(CHUNK, cap - c0)
                        nc.tensor.matmul(
                            out=ps[:, c0:c0 + w],
                            lhsT=lhs_sb[0:KDIM, tcol],
                            rhs=rhs_sb[:, sboff + c0:sboff + c0 + w],
                            start=True, stop=True,
                            tile_position=(0, 0),
                        )
                    nc.vector.max(out=s8_all[:, s * KNN:(s + 1) * KNN],
                                  in_=ps[:, :cap])
                    sboff += cap
                i = j
            if not lhs_rest_sent and l0 < npc:
                nc.sync.dma_start(out=lhs_sb[0:KDIM, l0:], in_=lhsT_d[:, l0:])
                nc.scalar.dma_start(out=lhs_sb[32:32 + KDIM, l0:],
                                    in_=lhsT_d[:, l0:])
                lhs_rest_sent = True
            epi_done = i
            epilogue(0, epi_done)

            # Big slots: per-group dual-ring DMAs, chunks alternate PE
            # row-groups 0/32 (DRAM group block column-swizzled
            # [even chunks | odd chunks]).
            for s in range(i, rt):
                cap = caps[s]
                ngroups = (cap + GROUP - 1) // GROUP
                cands = None
                if ngroups > 1:
                    cands = candp.tile([128, ngroups * KNN], dt.float32,
                                       tag="cands")
                tcol = slice(s * TILE, (s + 1) * TILE)
                for g in range(ngroups):
                    g0 = g * GROUP
                    gw = min(GROUP, cap - g0)
                    widths = [min(CHUNK, gw - j * CHUNK)
                              for j in range((gw + CHUNK - 1) // CHUNK)]
                    ev_w = sum(w for j, w in enumerate(widths) if j % 2 == 0)
                    od_w = gw - ev_w
                    rhs_sb = rhsp.tile([64, GROUP], dt.float16, tag="rhs")
                    base = offs[s] + g0
                    nc.sync.dma_start(out=rhs_sb[0:KDIM, :ev_w],
                                      in_=rhs_d[:, base:base + ev_w])
                    if od_w:
                        nc.scalar.dma_start(
                            out=rhs_sb[32:32 + KDIM, :od_w],
                            in_=rhs_d[:, base + ev_w:base + gw])
                    ps = psump.tile([128, GROUP], dt.float32, tag="ps")
                    for j, w in enumerate(widths):
                        p = 32 * (j % 2)
                        sb0 = (j // 2) * CHUNK
                        nc.tensor.matmul(
                            out=ps[:, j * CHUNK:j * CHUNK + w],
                            lhsT=lhs_sb[p:p + KDIM, tcol],
                            rhs=rhs_sb[p:p + KDIM, sb0:sb0 + w],
                            start=True, stop=True,
                            tile_position=(p, 0),
                        )
                    dst = (s8_all[:, s * KNN:(s + 1) * KNN] if ngroups == 1
                           else cands[:, g * KNN:(g + 1) * KNN])
                    nc.vector.max(out=dst, in_=ps[:, :gw])
                if ngroups > 1:
                    nc.vector.max(out=s8_all[:, s * KNN:(s + 1) * KNN],
                                  in_=cands[:, :])
            if epi_done < rt:
                epilogue(epi_done, rt)

    nc.compile()
    _CACHE[key] = nc
    return nc


def _run(to_filter, target_coords, trace=False):
    from concourse import bass_utils

    in_maps, rows_per_core, caps = _prep_pruned(to_filter, target_coords)
    nc = _build_pruned(caps)
    res = bass_utils.run_bass_kernel_spmd(
        nc, in_maps, core_ids=list(range(N_CORES)), trace=trace,
    )
    rt = len(caps)
    out = np.empty((N, KNN), np.float32)
    for c in range(N_CORES):
        oc = res.results[c]["out"].reshape(128, rt, KNN)
        for i, qs in enumerate(rows_per_core[c]):
            if len(qs):
                out[qs] = oc[:len(qs), i, :]
    return out, res


def kernel(to_filter, target_coords):
    out, _ = _run(to_filter, target_coords)
    return out
